# revision 1
# baseline (speedup 1.0000x reference)
"""HAN (hypergraph attention network) Trainium2 kernel.

Data-parallel over batch: 8 cores x 16 batch elements each, all params
replicated. Per-core pipeline per batch element b:
  gather emb rows (indirect DMA) -> PE-transpose to feature-major ->
  fp32r projections (hq/hs) -> bilinear logits (heads*queries = 128
  partitions) -> softmax (shift-invariance: h2att_b dropped) ->
  bf16 attention-value matmul -> pooled -> fc -> candidate sim ->
  log_softmax.
"""

import numpy as np
import ml_dtypes
from contextlib import ExitStack

import concourse.bass as bass
import concourse.bacc as bacc
import concourse.tile as tile
from concourse import mybir
from concourse.bass_utils import run_bass_kernel_spmd

F32 = mybir.dt.float32
F32R = mybir.dt.float32r
BF16 = mybir.dt.bfloat16
I32 = mybir.dt.int32
AF = mybir.ActivationFunctionType
ALU = mybir.AluOpType
AX = mybir.AxisListType

# Problem shapes (hardcoded per contract)
NCORES = 8
B = 128
BPC = B // NCORES          # 16 batch elems per core
NQ, NS, NODES = 16, 256, 3
V, E = 50000, 300
K = E * NODES              # 900 contraction dim for projections
C, H, OUT, NA = 1024, 8, 300, 5000
KC = 8                     # ceil(900/128) k-chunks; last has 4 rows
KR = [128] * 7 + [4]
CC = C // 128              # 8 c-chunks
OCN = [128, 128, 44]       # OUT=300 -> 3 o-chunks
SIMCH = [512] * 9 + [392]  # NA=5000 N-chunks

_CACHED = None


def _emit(ctx, tc, ins, outs):
    nc = tc.nc

    emb = ins["emb"]
    kg_idx = ins["kg_idx"]      # [BPC*2, 128, 3] i32
    q_idx = ins["q_idx"]        # [2, 128, 3] i32
    kwT_d = ins["kwT"]          # [128, KC*1024] f32
    qwT_d = ins["qwT"]
    h2aT_d = ins["h2aT"]        # [128, CC*H] f32
    kb_d = ins["kb"]            # [128, CC] f32
    qb_d = ins["qb"]
    fcb_d = ins["fcb"]          # [128, 3] f32
    sel1_d = ins["sel1"]        # [128, H] f32
    sel2_d = ins["sel2"]        # [H, 128] f32
    id32_d = ins["id32"]        # [128, 128] f32
    idbf_d = ins["idbf"]        # [128, 128] bf16
    fcwT_d = ins["fcwT"]        # [128, 64*OUT] bf16
    gloT_d = ins["gloT"]        # [3, 128, NA] bf16
    out_d = outs["out"]         # [BPC, NA] f32

    const = ctx.enter_context(tc.tile_pool(name="const", bufs=1))
    idxp = ctx.enter_context(tc.tile_pool(name="idxp", bufs=4))
    actp = ctx.enter_context(tc.tile_pool(name="actp", bufs=3))
    katp = ctx.enter_context(tc.tile_pool(name="katp", bufs=2))
    hstp = ctx.enter_context(tc.tile_pool(name="hstp", bufs=2))
    hsbp = ctx.enter_context(tc.tile_pool(name="hsbp", bufs=2))
    xtp = ctx.enter_context(tc.tile_pool(name="xtp", bufs=2))
    attp = ctx.enter_context(tc.tile_pool(name="attp", bufs=2))
    tmpp = ctx.enter_context(tc.tile_pool(name="tmpp", bufs=2))
    smlp = ctx.enter_context(tc.tile_pool(name="smlp", bufs=2))
    fwsp = ctx.enter_context(tc.tile_pool(name="fwsp", bufs=4))
    glsp = ctx.enter_context(tc.tile_pool(name="glsp", bufs=4))

    pstr = ctx.enter_context(tc.tile_pool(name="pstr", bufs=3, space="PSUM"))
    pspj = ctx.enter_context(tc.tile_pool(name="pspj", bufs=3, space="PSUM"))
    psyt = ctx.enter_context(tc.tile_pool(name="psyt", bufs=2, space="PSUM"))
    pssm = psyt  # merged: denom/sim tiles share the YT pool's banks

    # ---- resident weights/constants ----
    kwT = const.tile([128, KC * 1024], F32R, tag="kwT")
    nc.sync.dma_start(kwT[:], kwT_d[:])
    qwT = const.tile([128, KC * 1024], F32R, tag="qwT")
    nc.sync.dma_start(qwT[:], qwT_d[:])
    h2aT = const.tile([128, CC * H], F32, tag="h2aT")
    nc.sync.dma_start(h2aT[:], h2aT_d[:])
    kb = const.tile([128, CC], F32, tag="kb")
    nc.sync.dma_start(kb[:], kb_d[:])
    qb = const.tile([128, CC], F32, tag="qb")
    nc.sync.dma_start(qb[:], qb_d[:])
    fcb = const.tile([128, 3], F32, tag="fcb")
    nc.sync.dma_start(fcb[:], fcb_d[:])
    sel1 = const.tile([128, H], F32, tag="sel1")
    nc.sync.dma_start(sel1[:], sel1_d[:])
    sel2 = const.tile([H, 128], F32, tag="sel2")
    nc.sync.dma_start(sel2[:], sel2_d[:])
    id32 = const.tile([128, 128], F32R, tag="id32")
    nc.sync.dma_start(id32[:], id32_d[:])
    idbf = const.tile([128, 128], BF16, tag="idbf")
    nc.sync.dma_start(idbf[:], idbf_d[:])

    hqT = const.tile([128, CC * 256], F32, tag="hqT")      # [c, b*16+q]
    POOL = const.tile([128, CC * BPC * H], F32, tag="POOL")  # col cc*128+b*8+h
    POOLb = const.tile([128, CC * BPC * H], BF16, tag="POOLb")
    fcout = const.tile([128, 3 * BPC], BF16, tag="fcout")
    sim_sb = const.tile([BPC, NA], F32, tag="sim_sb")
    parti = const.tile([BPC, 16], F32, tag="parti")
    lse = const.tile([BPC, 1], F32, tag="lse")
    tot = const.tile([BPC, 1], F32, tag="tot")

    def gather_act(idx_dram_slice):
        """idx [128,6] -> act tile [128, 1800] f32r: 2 tokens per partition
        (token p in cols 0:900, token p+128 in cols 900:1800) -- halves the
        indirect-DMA count on the single SWDGE queue."""
        it = idxp.tile([128, 6], I32, tag="idx")
        nc.sync.dma_start(it[:], idx_dram_slice)
        at = actp.tile([128, 2 * K], F32R, tag="act")
        nc.gpsimd.indirect_dma_start(
            out=at[:],
            out_offset=None,
            in_=emb[:],
            in_offset=bass.IndirectOffsetOnAxis(ap=it[:, 0:6], axis=0),
        )
        return at

    def transpose_pack(at, dstT, tcol, copy_engine, acol=0):
        """act tile cols acol:acol+900 -> dstT[:, kc*ntok + tcol : +128].

        Groups: kc 0-3 into one psum bank, kc 4-6 + kc7(4 rows) into another.
        """
        dview = dstT[:].rearrange("p (k t) -> p k t", k=KC)
        at = at[:, acol: acol + K]
        ps_a = pstr.tile([128, 512], F32, tag="trps")
        for i, kc in enumerate(range(0, 4)):
            nc.tensor.transpose(
                out=ps_a[:, i * 128:(i + 1) * 128].bitcast(F32R),
                in_=at[:, kc * 128: kc * 128 + 128],
                identity=id32[:],
            )
        copy_engine(
            out=dview[:, 0:4, tcol:tcol + 128],
            in_=ps_a[:].rearrange("p (k t) -> p k t", k=4),
        )
        ps_b = pstr.tile([128, 512], F32, tag="trps")
        for i, kc in enumerate(range(4, 7)):
            nc.tensor.transpose(
                out=ps_b[:, i * 128:(i + 1) * 128].bitcast(F32R),
                in_=at[:, kc * 128: kc * 128 + 128],
                identity=id32[:],
            )
        nc.tensor.transpose(
            out=ps_b[0:4, 384:512].bitcast(F32R),
            in_=at[:, 896:900],
            identity=id32[:],
        )
        copy_engine(
            out=dview[:, 4:7, tcol:tcol + 128],
            in_=ps_b[:, 0:384].rearrange("p (k t) -> p k t", k=3),
        )
        copy_engine(
            out=dview[0:4, 7:8, tcol:tcol + 128],
            in_=ps_b[0:4, 384:512].unsqueeze(1),
        )

    def project(wT, actT, bias, dstT, ntok):
        """dstT[:, cc*ntok : +ntok] (f32r, c-chunk-major) = wT.T @ actT + bias"""
        for cc in range(CC):
            ps = pspj.tile([128, 512], F32, tag="pjps")
            for kc in range(KC):
                kr = KR[kc]
                nc.tensor.matmul(
                    out=ps[:, 0:ntok],
                    lhsT=wT[0:kr, kc * 1024 + cc * 128: kc * 1024 + cc * 128 + 128],
                    rhs=actT[0:kr, kc * ntok: kc * ntok + ntok],
                    start=(kc == 0),
                    stop=(kc == KC - 1),
                )
            nc.vector.tensor_scalar_add(
                dstT[:, cc * ntok:(cc + 1) * ntok], ps[:, 0:ntok],
                bias[:, cc: cc + 1])

    # ---- prologue: hq for all 16 b at once (256 ques tokens) ----
    qactT = const.tile([128, KC * 256], F32R, tag="qactT")
    at = gather_act(q_idx[0])
    transpose_pack(at, qactT, 0, nc.scalar.copy, acol=0)
    transpose_pack(at, qactT, 128, nc.scalar.copy, acol=K)
    project(qwT, qactT, qb, hqT, 256)

    hqv = hqT[:].rearrange("p (c t) -> p c t", c=CC)  # [128, 8, 256]
    h2av = h2aT[:].rearrange("p (c h) -> p c h", c=CC)  # [128, 8, 8]
    pv = POOL[:].rearrange("p (c b h) -> p c b h", c=CC, b=BPC)

    # ---- per pair of batch elements: gather+transpose+project at N=512 ----
    for bp in range(BPC // 2):
        kactT = katp.tile([128, KC * 512], F32R, tag="kactT", bufs=1)
        for g in range(2):
            at = gather_act(kg_idx[2 * bp + g])
            transpose_pack(at, kactT, g * 256, nc.scalar.copy, acol=0)
            transpose_pack(at, kactT, g * 256 + 128, nc.scalar.copy, acol=K)

        hsTp = hstp.tile([128, CC * 512], F32R, tag="hsT", bufs=1)
        project(kwT, kactT, kb, hsTp, 512)

        for half in range(2):
            b = bp * 2 + half
            hb = half * 256  # this b's token offset inside the pair

            # hs token-major bf16: [s-chunk partitions, col st*1024 + c]
            hs_sb = hsbp.tile([128, 2 * 1024], BF16, tag="hs_sb")
            for st in range(2):
                ps = pstr.tile([128, 512], F32, tag="trps")
                for i, cc in enumerate(range(0, 4)):
                    nc.tensor.transpose(
                        out=ps[:, i * 128:(i + 1) * 128].bitcast(F32R),
                        in_=hsTp[:, cc * 512 + hb + st * 128: cc * 512 + hb + st * 128 + 128],
                        identity=id32[:],
                    )
                nc.scalar.copy(out=hs_sb[:, st * 1024: st * 1024 + 512], in_=ps[:])
                ps2 = pstr.tile([128, 512], F32, tag="trps")
                for i, cc in enumerate(range(4, 8)):
                    nc.tensor.transpose(
                        out=ps2[:, i * 128:(i + 1) * 128].bitcast(F32R),
                        in_=hsTp[:, cc * 512 + hb + st * 128: cc * 512 + hb + st * 128 + 128],
                        identity=id32[:],
                    )
                nc.scalar.copy(out=hs_sb[:, st * 1024 + 512: st * 1024 + 1024],
                               in_=ps2[:])

            # X^T[c, h*16+q] = hqT[c, q] * h2aT[c, h]  (one grouped DVE op)
            XT = xtp.tile([128, 1024], F32R, tag="XT")
            nc.vector.tensor_tensor(
                out=XT[:].rearrange("p (c h q) -> p c h q", c=CC, h=H),
                in0=hqv[:, :, b * 16: b * 16 + 16].unsqueeze(2).to_broadcast(
                    [128, CC, H, 16]),
                in1=h2av[:, :, :].unsqueeze(3).to_broadcast([128, CC, H, 16]),
                op=ALU.mult,
            )

            # logits[hq=128, s=256]
            plg = pspj.tile([128, 512], F32, tag="pjps")
            for cc in range(CC):
                nc.tensor.matmul(
                    out=plg[:, 0:256],
                    lhsT=XT[:, cc * 128: cc * 128 + 128],
                    rhs=hsTp[:, cc * 512 + hb: cc * 512 + hb + 256],
                    start=(cc == 0),
                    stop=(cc == CC - 1),
                )

            # softmax over flat (q,s) per (b,h); logits tiny -> skip max-sub
            att = attp.tile([128, 256], BF16, tag="att")
            qsum = smlp.tile([128, 1], F32, tag="qsum")
            nc.scalar.activation(att[:], plg[:, 0:256], AF.Exp, accum_out=qsum[:])

            dps = pssm.tile([128, 512], F32, tag="ytps", name="dps")
            nc.tensor.matmul(out=dps[0:8, 0:1], lhsT=sel1[:], rhs=qsum[:],
                             start=True, stop=True)
            r8 = smlp.tile([8, 1], F32, tag="r8")
            nc.vector.reciprocal(r8[:], dps[0:8, 0:1])
            nc.tensor.matmul(out=dps[:, 1:2], lhsT=sel2[:], rhs=r8[:],
                             start=True, stop=True)
            rsb = smlp.tile([128, 1], F32, tag="rsb")
            nc.vector.tensor_copy(rsb[:], dps[:, 1:2])

            attn = attp.tile([128, 256], BF16, tag="attn")
            nc.vector.tensor_scalar_mul(attn[:], att[:], rsb[:])

            # attT [s, hq] bf16
            attT = attp.tile([128, 256], BF16, tag="attT")
            psTb = pstr.tile([128, 256], BF16, tag="trps")
            for st in range(2):
                nc.tensor.transpose(
                    out=psTb[:, st * 128:(st + 1) * 128],
                    in_=attn[:, st * 128:(st + 1) * 128],
                    identity=idbf[:],
                )
            nc.vector.tensor_copy(attT[:], psTb[:])

            # YT[c, hq] per c-chunk; pooled[h,c] = sum_q hqT * sum_s attT*hs
            for ccg in range(2):
                py = psyt.tile([128, 512], F32, tag="ytps")
                for i in range(4):
                    cc = ccg * 4 + i
                    for st in range(2):
                        nc.tensor.matmul(
                            out=py[:, i * 128:(i + 1) * 128],
                            lhsT=hs_sb[:, st * 1024 + cc * 128: st * 1024 + cc * 128 + 128],
                            rhs=attT[:, st * 128:(st + 1) * 128],
                            start=(st == 0),
                            stop=(st == 1),
                        )
                tmp = tmpp.tile([128, 512], F32, tag="tmp")
                nc.vector.tensor_tensor(
                    out=tmp[:].rearrange("p (c h q) -> p c h q", c=4, h=H),
                    in0=py[:].rearrange("p (c h q) -> p c h q", c=4, h=H),
                    in1=hqv[:, ccg * 4:(ccg + 1) * 4, b * 16: b * 16 + 16].unsqueeze(2).to_broadcast([128, 4, H, 16]),
                    op=ALU.mult,
                )
                nc.vector.reduce_sum(
                    out=pv[:, ccg * 4:(ccg + 1) * 4, b, :],
                    in_=tmp[:].rearrange("p (c h q) -> p c h q", c=4, h=H),
                    axis=AX.X,
                )

    # ---- fc: out[o, b] = sum_{h,c} fc_w[o, h*1024+c] * pooled ----
    nc.vector.tensor_copy(POOLb[:], POOL[:])
    poolv = POOLb[:].rearrange("p (c b h) -> p c b h", c=CC, b=BPC)
    # 3 accumulators in 3 different PSUM banks (concurrent open groups
    # in one bank are illegal); pools are otherwise idle in this phase.
    pfc = [pspj.tile([128, 512], F32, tag="pjps", name="pfc0"),
           pstr.tile([128, 512], F32, tag="trps", name="pfc1"),
           psyt.tile([128, 512], F32, tag="ytps", name="pfc2")]
    nhc = H * CC
    for h in range(H):
        for cc in range(CC):
            i = h * CC + cc
            fw = fwsp.tile([128, OUT], BF16, tag="fw")
            nc.sync.dma_start(fw[:], fcwT_d[:, i * OUT:(i + 1) * OUT])
            for oc in range(3):
                ocn = OCN[oc]
                nc.tensor.matmul(
                    out=pfc[oc][0:ocn, 0:16],
                    lhsT=fw[:, oc * 128: oc * 128 + ocn],
                    rhs=poolv[:, cc, :, h],
                    start=(i == 0),
                    stop=(i == nhc - 1),
                )
    for oc in range(3):
        ocn = OCN[oc]
        nc.scalar.activation(
            out=fcout[0:ocn, oc * 16: oc * 16 + 16],
            in_=pfc[oc][0:ocn, 0:16],
            func=AF.Identity,
            bias=fcb[0:ocn, oc: oc + 1],
        )

    # ---- sim = fcout.T @ gloveT ; log_softmax over NA ----
    a0 = 0
    for ci, n in enumerate(SIMCH):
        pss = pssm.tile([16, 512], F32, tag="ytps", name="pss")
        for oc in range(3):
            ocn = OCN[oc]
            gs = glsp.tile([128, 512], BF16, tag="gs")
            nc.sync.dma_start(gs[0:ocn, 0:n], gloT_d[oc, 0:ocn, a0: a0 + n])
            nc.tensor.matmul(
                out=pss[0:16, 0:n],
                lhsT=fcout[0:ocn, oc * 16: oc * 16 + 16],
                rhs=gs[0:ocn, 0:n],
                start=(oc == 0),
                stop=(oc == 2),
            )
        junk = tmpp.tile([128, 512], F32, tag="tmp")
        nc.scalar.activation(junk[0:16, 0:n], pss[0:16, 0:n], AF.Exp,
                             accum_out=parti[:, ci: ci + 1])
        nc.vector.tensor_copy(sim_sb[:, a0: a0 + n], pss[0:16, 0:n])
        a0 += n

    nc.vector.reduce_sum(out=tot[:], in_=parti[:, 0:10], axis=AX.X)
    nc.scalar.activation(lse[:], tot[:], AF.Ln)
    nc.vector.tensor_scalar_sub(sim_sb[:], sim_sb[:], lse[:])
    nc.sync.dma_start(out_d[:], sim_sb[:])


def _build():
    nc = bacc.Bacc("TRN2", target_bir_lowering=False, debug=False,
                   num_devices=NCORES)
    ins = {}

    def di(name, shape, dtype):
        ins[name] = nc.dram_tensor(name, list(shape), dtype,
                                   kind="ExternalInput").ap()

    di("emb", (V, E), F32R)
    di("kg_idx", (BPC, 128, 6), I32)
    di("q_idx", (1, 128, 6), I32)
    di("kwT", (128, KC * 1024), F32R)
    di("qwT", (128, KC * 1024), F32R)
    di("h2aT", (128, CC * H), F32)
    di("kb", (128, CC), F32)
    di("qb", (128, CC), F32)
    di("fcb", (128, 3), F32)
    di("sel1", (128, H), F32)
    di("sel2", (H, 128), F32)
    di("id32", (128, 128), F32R)
    di("idbf", (128, 128), BF16)
    di("fcwT", (128, H * CC * OUT), BF16)
    di("gloT", (3, 128, NA), BF16)
    outs = {"out": nc.dram_tensor("out", [BPC, NA], F32,
                                  kind="ExternalOutput").ap()}

    with tile.TileContext(nc) as tc, ExitStack() as ctx:
        _emit(ctx, tc, ins, outs)
    nc.compile()
    return nc


def _trunc22(a):
    """Truncate f32 -> FP22 bits (e8m13): what the PE reads in fp32r mode."""
    b = np.ascontiguousarray(a, np.float32).view(np.uint32) & np.uint32(0xFFFFFC00)
    return b.view(np.float32)


def _pack_host(emb, q2h_w, q2h_b, k2h_w, k2h_b, h2att_w, fc_w, fc_b,
               glove_cands):
    """One-time layout prep of replicated params (host numpy)."""
    f32 = np.float32
    bf = ml_dtypes.bfloat16

    def packT(W):  # [C, 900] -> [128, KC*1024]: col kc*1024+c, row = k in chunk
        P = np.zeros((128, KC * 1024), f32)
        for kc in range(KC):
            kr = KR[kc]
            P[0:kr, kc * 1024:(kc + 1) * 1024] = W[:, kc * 128: kc * 128 + kr].T
        return P

    kwT = _trunc22(packT(np.asarray(k2h_w, f32)))
    qwT = _trunc22(packT(np.asarray(q2h_w, f32)))

    h2aT = np.zeros((128, CC * H), f32)
    for cc in range(CC):
        h2aT[:, cc * H:(cc + 1) * H] = np.asarray(h2att_w, f32)[:, cc * 128:(cc + 1) * 128].T

    kb = np.asarray(k2h_b, f32).reshape(CC, 128).T.copy()
    qb = np.asarray(q2h_b, f32).reshape(CC, 128).T.copy()

    fcb = np.zeros((128, 3), f32)
    fcb_src = np.asarray(fc_b, f32)
    for oc in range(3):
        fcb[0:OCN[oc], oc] = fcb_src[oc * 128: oc * 128 + OCN[oc]]

    sel1 = np.zeros((128, H), f32)
    for p in range(128):
        sel1[p, p // 16] = 1.0
    sel2 = np.ascontiguousarray(sel1.T)

    id32 = np.eye(128, dtype=f32)  # exact in FP22
    idbf = np.eye(128, dtype=bf)

    # fc_w [OUT, H*C]: pack col (h*CC+cc)*OUT + o = fc_w[o, h*1024+cc*128+p]
    fcw = np.asarray(fc_w, f32).reshape(OUT, H, CC, 128)
    fcwT = np.ascontiguousarray(
        fcw.transpose(3, 1, 2, 0).reshape(128, H * CC * OUT)).astype(bf)

    glo = np.asarray(glove_cands, f32)  # [NA, OUT]
    gloT = np.zeros((3, 128, NA), f32)
    for oc in range(3):
        gloT[oc, 0:OCN[oc], :] = glo[:, oc * 128: oc * 128 + OCN[oc]].T
    gloT = gloT.astype(bf)

    return dict(kwT=kwT, qwT=qwT, h2aT=h2aT, kb=kb, qb=qb, fcb=fcb,
                sel1=sel1, sel2=sel2, id32=id32, idbf=idbf, fcwT=fcwT,
                gloT=gloT)


def make_in_maps(he_ques, he_kg, emb, q2h_w, q2h_b, k2h_w, k2h_b,
                 h2att_w, h2att_b, fc_w, fc_b, glove_cands):
    emb = _trunc22(np.asarray(emb, np.float32))
    shared = _pack_host(emb, q2h_w, q2h_b, k2h_w, k2h_b, h2att_w, fc_w,
                        fc_b, glove_cands)
    shared["emb"] = emb
    he_kg = np.asarray(he_kg).astype(np.int32)
    he_ques = np.asarray(he_ques).astype(np.int32)
    maps = []
    for c in range(NCORES):
        # [tok, 3] -> [ntile, 2, 128, 3] -> [ntile, 128, 6]: token p and
        # p+128 of each 256-token block share partition p (cols 0:3 / 3:6)
        kgc = (he_kg[c * BPC:(c + 1) * BPC].reshape(BPC, 2, 128, 3)
               .transpose(0, 2, 1, 3).reshape(BPC, 128, 6))
        quc = (he_ques[c * BPC:(c + 1) * BPC].reshape(1, 2, 128, 3)
               .transpose(0, 2, 1, 3).reshape(1, 128, 6))
        m = dict(shared)
        m["kg_idx"] = np.ascontiguousarray(kgc)
        m["q_idx"] = np.ascontiguousarray(quc)
        maps.append(m)
    return maps


def kernel(**inputs):
    global _CACHED
    if _CACHED is None:
        _CACHED = _build()
    nc = _CACHED
    in_maps = make_in_maps(**inputs)
    res = run_bass_kernel_spmd(nc, in_maps, list(range(NCORES)))
    return np.concatenate([r["out"] for r in res.results], axis=0)



# revision 2
# speedup vs baseline: 2.2000x; 2.2000x over previous
"""HAN (hypergraph attention network) Trainium2 kernel, v2.

Data-parallel over batch: 8 cores x 16 batch elements, params replicated.
v2 pipeline: per-core vocabulary compaction (int16 idx) + bf16 padded
embedding table with a built-in ones column -> dma_gather(transpose=True)
lands activations k-major with zero PE transposes -> bf16 projections with
bias folded into the weight row for the ones column -> bilinear attention
(heads*queries = 128 partitions) -> softmax -> bf16 attention-value matmul
-> pooled -> fc -> candidate sim -> log_softmax.  fc/glove weights are
loaded as a few large resident tiles so the tail phases never stall on DMA.
"""

import numpy as np
import ml_dtypes
from contextlib import ExitStack

import concourse.bass as bass
import concourse.bacc as bacc
import concourse.tile as tile
from concourse import mybir
from concourse.bass_utils import run_bass_kernel_spmd

F32 = mybir.dt.float32
BF16 = mybir.dt.bfloat16
I16 = mybir.dt.int16
AF = mybir.ActivationFunctionType
ALU = mybir.AluOpType
AX = mybir.AxisListType

NCORES = 8
B = 128
BPC = B // NCORES          # 16 batch elems per core
NQ, NS, NODES = 16, 256, 3
V, E = 50000, 300
ES = 384                   # padded emb row (bf16) -> 768B, %256==0
UMAX = 13056               # per-core unique rows cap: 16*(256+16)*3 / ... hard bound
C, H, OUT, NA = 1024, 8, 300, 5000
CC = C // 128              # 8 c-chunks
NCH = 9                    # (node j, k-chunk c) pairs: 3x3
OCN = [128, 128, 44]       # OUT=300 -> 3 o-chunks
SIMCH = [512] * 9 + [392]  # NA=5000 N-chunks
FCT = 4                    # fcw resident tiles
FCC = H * CC // FCT        # (h,cc) chunks per fcw tile

_CACHED = None


def _emit(ctx, tc, ins, outs):
    nc = tc.nc

    emb = ins["emb"]            # [UMAX, ES] bf16
    kg_idx = ins["kg_idx"]      # [128, 8*96] i16 (pair-major)
    q_idx = ins["q_idx"]        # [128, 48] i16
    kwT_d = ins["kwT"]          # [128, NCH*1024] bf16
    qwT_d = ins["qwT"]
    h2aT_d = ins["h2aT"]        # [128, CC*H] f32
    fcb_d = ins["fcb"]          # [128, 3] f32
    sel1_d = ins["sel1"]        # [128, H] f32
    sel2_d = ins["sel2"]        # [H, 128] f32
    idbf_d = ins["idbf"]        # [128, 128] bf16
    fcwT_d = ins["fcwT"]        # [128, H*CC*OUT] bf16
    gloT_d = ins["gloT"]        # [3, 128, NA] bf16
    out_d = outs["out"]         # [BPC, NA] f32

    const = ctx.enter_context(tc.tile_pool(name="const", bufs=1))
    actp = ctx.enter_context(tc.tile_pool(name="actp", bufs=2))
    hstp = ctx.enter_context(tc.tile_pool(name="hstp", bufs=2))
    hsbp = ctx.enter_context(tc.tile_pool(name="hsbp", bufs=2))
    xtp = ctx.enter_context(tc.tile_pool(name="xtp", bufs=2))
    attp = ctx.enter_context(tc.tile_pool(name="attp", bufs=2))
    tmpp = ctx.enter_context(tc.tile_pool(name="tmpp", bufs=2))
    smlp = ctx.enter_context(tc.tile_pool(name="smlp", bufs=2))

    pstr = ctx.enter_context(tc.tile_pool(name="pstr", bufs=3, space="PSUM"))
    pspj = ctx.enter_context(tc.tile_pool(name="pspj", bufs=3, space="PSUM"))
    psyt = ctx.enter_context(tc.tile_pool(name="psyt", bufs=2, space="PSUM"))

    # ---- resident constants / weights ----
    itq = const.tile([128, 48], I16, tag="itq")
    nc.sync.dma_start(itq[:], q_idx[:])
    itk = const.tile([128, 8 * 96], I16, tag="itk")
    nc.sync.dma_start(itk[:], kg_idx[:])
    qwT = const.tile([128, NCH * 1024], BF16, tag="qwT")
    nc.sync.dma_start(qwT[:], qwT_d[:])
    kwT = const.tile([128, NCH * 1024], BF16, tag="kwT")
    nc.sync.dma_start(kwT[:], kwT_d[:])
    h2aT = const.tile([128, CC * H], F32, tag="h2aT")
    nc.sync.dma_start(h2aT[:], h2aT_d[:])
    fcb = const.tile([128, 3], F32, tag="fcb")
    nc.sync.dma_start(fcb[:], fcb_d[:])
    sel1 = const.tile([128, H], F32, tag="sel1")
    nc.sync.dma_start(sel1[:], sel1_d[:])
    sel2 = const.tile([H, 128], F32, tag="sel2")
    nc.sync.dma_start(sel2[:], sel2_d[:])
    idbf = const.tile([128, 128], BF16, tag="idbf")
    nc.sync.dma_start(idbf[:], idbf_d[:])
    fcw_sb = []
    for t in range(FCT):
        fw = const.tile([128, FCC * OUT], BF16, tag=f"fcw{t}")
        nc.sync.dma_start(fw[:], fcwT_d[:, t * FCC * OUT:(t + 1) * FCC * OUT])
        fcw_sb.append(fw)
    glo_sb = []
    for oc in range(3):
        gs = const.tile([128, NA], BF16, tag=f"glo{oc}")
        nc.sync.dma_start(gs[:], gloT_d[oc])
        glo_sb.append(gs)

    hqT = const.tile([128, CC * 256], F32, tag="hqT")      # [c, b*16+q]
    POOL = const.tile([128, CC * BPC * H], F32, tag="POOL")  # col cc*128+b*8+h
    POOLb = const.tile([128, CC * BPC * H], BF16, tag="POOLb")
    fcout = const.tile([128, 3 * BPC], BF16, tag="fcout")
    sim_sb = const.tile([BPC, NA], F32, tag="sim_sb")
    parti = const.tile([BPC, 16], F32, tag="parti")
    lse = const.tile([BPC, 1], F32, tag="lse")
    tot = const.tile([BPC, 1], F32, tag="tot")

    def gather(idx_slice, ntok):
        """ntok tokens x NODES rows, j-major idx order; num_idxs > 768
        crashes the gather ucode, so one 768-idx gather per contiguous
        [3, 768] block. Returns list of [128, 3, 768] views."""
        ni = NODES * ntok
        ng = ni // 768
        at = actp.tile([128, 3 * ni], BF16, tag="act")
        views = []
        for g in range(ng):
            v = at[:, g * 2304:(g + 1) * 2304].rearrange(
                "p (c n) -> p c n", c=3)
            nc.gpsimd.dma_gather(
                out_ap=v,
                in_ap=emb[:],
                idxs_ap=idx_slice[:, g * 48:(g + 1) * 48],
                num_idxs=768,
                num_idxs_reg=768,
                elem_size=ES,
                transpose=True,
            )
            views.append(v)
        return views

    def project(wT, atvs, dstT, ntok):
        """dstT[:, cc*ntok : +ntok] = wT.T @ act (+bias via ones column).

        rhs pieces per (node j, k-chunk c): with one gather (ntok=256) the
        j-blocks are whole; with two (ntok=512) node 1 straddles the two
        gather blocks, so it contributes two half-range matmuls."""
        if len(atvs) == 1:
            pieces = [(j * 3 + c, atvs[0][:, c, j * ntok:(j + 1) * ntok], 0, ntok)
                      for j in range(3) for c in range(3)]
        else:
            pieces = (
                [(c, atvs[0][:, c, 0:512], 0, 512) for c in range(3)]
                + [(3 + c, atvs[0][:, c, 512:768], 0, 256) for c in range(3)]
                + [(3 + c, atvs[1][:, c, 0:256], 256, 512) for c in range(3)]
                + [(6 + c, atvs[1][:, c, 256:768], 0, 512) for c in range(3)]
            )
            # full-range ops must open and close the accumulation group
            pieces = pieces[0:3] + pieces[3:9] + pieces[9:12]
        for cc in range(CC):
            ps = pspj.tile([128, 512], F32, tag="pjps")
            for i, (ch, rhs, a, bnd) in enumerate(pieces):
                nc.tensor.matmul(
                    out=ps[:, a:bnd],
                    lhsT=wT[:, ch * 1024 + cc * 128: ch * 1024 + cc * 128 + 128],
                    rhs=rhs,
                    start=(i == 0),
                    stop=(i == len(pieces) - 1),
                )
            nc.scalar.copy(out=dstT[:, cc * ntok:(cc + 1) * ntok],
                           in_=ps[:, 0:ntok])

    # ---- prologue: hq for all 16 b (256 ques tokens) ----
    atq = gather(itq[:], 256)
    project(qwT, atq, hqT, 256)

    hqv = hqT[:].rearrange("p (c t) -> p c t", c=CC)  # [128, 8, 256]
    h2av = h2aT[:].rearrange("p (c h) -> p c h", c=CC)  # [128, 8, 8]
    pv = POOL[:].rearrange("p (c b h) -> p c b h", c=CC, b=BPC)

    # ---- per pair of batch elements ----
    for bp in range(BPC // 2):
        atk = gather(itk[:, bp * 96:(bp + 1) * 96], 512)

        hsT = hstp.tile([128, CC * 512], BF16, tag="hsT")
        project(kwT, atk, hsT, 512)

        for half in range(2):
            b = bp * 2 + half
            hb = half * 256

            # hs token-major bf16: [s-chunk partitions, col st*1024 + c]
            hs_sb = hsbp.tile([128, 2 * 1024], BF16, tag="hs_sb")
            for st in range(2):
                ps = pstr.tile([128, 1024], BF16, tag="trps")
                for cc in range(CC):
                    nc.tensor.transpose(
                        out=ps[:, cc * 128:(cc + 1) * 128],
                        in_=hsT[:, cc * 512 + hb + st * 128: cc * 512 + hb + st * 128 + 128],
                        identity=idbf[:],
                    )
                nc.scalar.copy(out=hs_sb[:, st * 1024:(st + 1) * 1024], in_=ps[:])

            # X^T[c, h*16+q] = hqT[c, q] * h2aT[c, h]
            XT = xtp.tile([128, 1024], BF16, tag="XT")
            nc.vector.tensor_tensor(
                out=XT[:].rearrange("p (c h q) -> p c h q", c=CC, h=H),
                in0=hqv[:, :, b * 16: b * 16 + 16].unsqueeze(2).to_broadcast(
                    [128, CC, H, 16]),
                in1=h2av[:, :, :].unsqueeze(3).to_broadcast([128, CC, H, 16]),
                op=ALU.mult,
            )

            # logits[hq=128, s=256]
            plg = pspj.tile([128, 512], F32, tag="pjps")
            for cc in range(CC):
                nc.tensor.matmul(
                    out=plg[:, 0:256],
                    lhsT=XT[:, cc * 128: cc * 128 + 128],
                    rhs=hsT[:, cc * 512 + hb: cc * 512 + hb + 256],
                    start=(cc == 0),
                    stop=(cc == CC - 1),
                )

            # softmax over flat (q,s) per (b,h); logits tiny -> skip max-sub
            att = attp.tile([128, 256], BF16, tag="att")
            qsum = smlp.tile([128, 1], F32, tag="qsum")
            nc.scalar.activation(att[:], plg[:, 0:256], AF.Exp, accum_out=qsum[:])

            dps = psyt.tile([128, 512], F32, tag="ytps", name="dps")
            nc.tensor.matmul(out=dps[0:8, 0:1], lhsT=sel1[:], rhs=qsum[:],
                             start=True, stop=True)
            r8 = smlp.tile([8, 1], F32, tag="r8")
            nc.vector.reciprocal(r8[:], dps[0:8, 0:1])
            nc.tensor.matmul(out=dps[:, 1:2], lhsT=sel2[:], rhs=r8[:],
                             start=True, stop=True)
            rsb = smlp.tile([128, 1], F32, tag="rsb")
            nc.vector.tensor_copy(rsb[:], dps[:, 1:2])

            attn = attp.tile([128, 256], BF16, tag="attn")
            nc.vector.tensor_scalar_mul(attn[:], att[:], rsb[:])

            # attT [s, hq] bf16
            attT = attp.tile([128, 256], BF16, tag="attT")
            psTb = pstr.tile([128, 256], BF16, tag="trps", name="psTb")
            for st in range(2):
                nc.tensor.transpose(
                    out=psTb[:, st * 128:(st + 1) * 128],
                    in_=attn[:, st * 128:(st + 1) * 128],
                    identity=idbf[:],
                )
            nc.vector.tensor_copy(attT[:], psTb[:])

            # YT[c, hq] per c-chunk; pooled[h,c] = sum_q hqT * sum_s attT*hs
            for ccg in range(2):
                py = psyt.tile([128, 512], F32, tag="ytps")
                for i in range(4):
                    cc = ccg * 4 + i
                    for st in range(2):
                        nc.tensor.matmul(
                            out=py[:, i * 128:(i + 1) * 128],
                            lhsT=hs_sb[:, st * 1024 + cc * 128: st * 1024 + cc * 128 + 128],
                            rhs=attT[:, st * 128:(st + 1) * 128],
                            start=(st == 0),
                            stop=(st == 1),
                        )
                tmp = tmpp.tile([128, 512], F32, tag="tmp")
                nc.vector.tensor_tensor(
                    out=tmp[:].rearrange("p (c h q) -> p c h q", c=4, h=H),
                    in0=py[:].rearrange("p (c h q) -> p c h q", c=4, h=H),
                    in1=hqv[:, ccg * 4:(ccg + 1) * 4, b * 16: b * 16 + 16].unsqueeze(2).to_broadcast([128, 4, H, 16]),
                    op=ALU.mult,
                )
                nc.vector.reduce_sum(
                    out=pv[:, ccg * 4:(ccg + 1) * 4, b, :],
                    in_=tmp[:].rearrange("p (c h q) -> p c h q", c=4, h=H),
                    axis=AX.X,
                )

    # ---- fc: out[o, b] = sum_{h,c} fc_w[o, h*1024+c] * pooled ----
    nc.vector.tensor_copy(POOLb[:], POOL[:])
    poolv = POOLb[:].rearrange("p (c b h) -> p c b h", c=CC, b=BPC)
    pfc = [pspj.tile([128, 512], F32, tag="pjps", name="pfc0"),
           pstr.tile([128, 512], F32, tag="trps", name="pfc1"),
           psyt.tile([128, 512], F32, tag="ytps", name="pfc2")]
    nhc = H * CC
    for h in range(H):
        for cc in range(CC):
            i = h * CC + cc
            fw = fcw_sb[i // FCC]
            fo = (i % FCC) * OUT
            for oc in range(3):
                ocn = OCN[oc]
                nc.tensor.matmul(
                    out=pfc[oc][0:ocn, 0:16],
                    lhsT=fw[:, fo + oc * 128: fo + oc * 128 + ocn],
                    rhs=poolv[:, cc, :, h],
                    start=(i == 0),
                    stop=(i == nhc - 1),
                )
    for oc in range(3):
        ocn = OCN[oc]
        nc.scalar.activation(
            out=fcout[0:ocn, oc * 16: oc * 16 + 16],
            in_=pfc[oc][0:ocn, 0:16],
            func=AF.Identity,
            bias=fcb[0:ocn, oc: oc + 1],
        )

    # ---- sim = fcout.T @ gloveT ; log_softmax over NA ----
    a0 = 0
    for ci, n in enumerate(SIMCH):
        pss = psyt.tile([16, 512], F32, tag="ytps", name="pss")
        for oc in range(3):
            ocn = OCN[oc]
            nc.tensor.matmul(
                out=pss[0:16, 0:n],
                lhsT=fcout[0:ocn, oc * 16: oc * 16 + 16],
                rhs=glo_sb[oc][0:ocn, a0: a0 + n],
                start=(oc == 0),
                stop=(oc == 2),
            )
        junk = tmpp.tile([128, 512], F32, tag="tmp")
        nc.scalar.activation(junk[0:16, 0:n], pss[0:16, 0:n], AF.Exp,
                             accum_out=parti[:, ci: ci + 1])
        nc.vector.tensor_copy(sim_sb[:, a0: a0 + n], pss[0:16, 0:n])
        a0 += n

    nc.vector.reduce_sum(out=tot[:], in_=parti[:, 0:10], axis=AX.X)
    nc.scalar.activation(lse[:], tot[:], AF.Ln)
    nc.vector.tensor_scalar_sub(sim_sb[:], sim_sb[:], lse[:])
    nc.sync.dma_start(out_d[:], sim_sb[:])


def _build():
    nc = bacc.Bacc("TRN2", target_bir_lowering=False, debug=False,
                   num_devices=NCORES)
    ins = {}

    def di(name, shape, dtype):
        ins[name] = nc.dram_tensor(name, list(shape), dtype,
                                   kind="ExternalInput").ap()

    di("emb", (UMAX, ES), BF16)
    di("kg_idx", (128, 8 * 96), I16)
    di("q_idx", (128, 48), I16)
    di("kwT", (128, NCH * 1024), BF16)
    di("qwT", (128, NCH * 1024), BF16)
    di("h2aT", (128, CC * H), F32)
    di("fcb", (128, 3), F32)
    di("sel1", (128, H), F32)
    di("sel2", (H, 128), F32)
    di("idbf", (128, 128), BF16)
    di("fcwT", (128, H * CC * OUT), BF16)
    di("gloT", (3, 128, NA), BF16)
    outs = {"out": nc.dram_tensor("out", [BPC, NA], F32,
                                  kind="ExternalOutput").ap()}

    with tile.TileContext(nc) as tc, ExitStack() as ctx:
        _emit(ctx, tc, ins, outs)
    nc.compile()
    return nc


def _pack_wT(W, bias):
    """[C, 900] f32 -> [128, NCH*1024] bf16 with bias folded at (0,2) row 44."""
    bf = ml_dtypes.bfloat16
    P = np.zeros((128, NCH * 1024), np.float32)
    for j in range(NODES):
        for c in range(3):
            ch = j * 3 + c
            kr = min(128, E - c * 128)
            P[0:kr, ch * 1024:(ch + 1) * 1024] = \
                W[:, j * E + c * 128: j * E + c * 128 + kr].T
    P[44, 2 * 1024:3 * 1024] = bias      # chunk (0,2) row 44 <- ones column
    return P.astype(bf)


def make_in_maps(he_ques, he_kg, emb, q2h_w, q2h_b, k2h_w, k2h_b,
                 h2att_w, h2att_b, fc_w, fc_b, glove_cands):
    f32 = np.float32
    bf = ml_dtypes.bfloat16
    emb = np.asarray(emb, f32)
    he_kg = np.asarray(he_kg).astype(np.int64)
    he_ques = np.asarray(he_ques).astype(np.int64)

    kwT = _pack_wT(np.asarray(k2h_w, f32), np.asarray(k2h_b, f32))
    qwT = _pack_wT(np.asarray(q2h_w, f32), np.asarray(q2h_b, f32))

    h2aT = np.zeros((128, CC * H), f32)
    for cc in range(CC):
        h2aT[:, cc * H:(cc + 1) * H] = np.asarray(h2att_w, f32)[:, cc * 128:(cc + 1) * 128].T

    fcb = np.zeros((128, 3), f32)
    fcb_src = np.asarray(fc_b, f32)
    for oc in range(3):
        fcb[0:OCN[oc], oc] = fcb_src[oc * 128: oc * 128 + OCN[oc]]

    sel1 = np.zeros((128, H), f32)
    for p in range(128):
        sel1[p, p // 16] = 1.0
    sel2 = np.ascontiguousarray(sel1.T)
    idbf = np.eye(128, dtype=bf)

    fcw = np.asarray(fc_w, f32).reshape(OUT, H, CC, 128)
    fcwT = np.ascontiguousarray(
        fcw.transpose(3, 1, 2, 0).reshape(128, H * CC * OUT)).astype(bf)

    glo = np.asarray(glove_cands, f32)
    gloT = np.zeros((3, 128, NA), f32)
    for oc in range(3):
        gloT[oc, 0:OCN[oc], :] = glo[:, oc * 128: oc * 128 + OCN[oc]].T
    gloT = gloT.astype(bf)

    shared = dict(kwT=kwT, qwT=qwT, h2aT=h2aT, fcb=fcb, sel1=sel1,
                  sel2=sel2, idbf=idbf, fcwT=fcwT, gloT=gloT)

    def wrap_idx(flat):
        """[n] -> [128, n//16] int16 wrapped in 16 partitions, replicated."""
        n = flat.shape[0]
        t = np.zeros((128, n // 16), np.int16)
        t[0:16] = flat.reshape(n // 16, 16).T
        for g in range(1, 8):
            t[g * 16:(g + 1) * 16] = t[0:16]
        return t

    maps = []
    for core in range(NCORES):
        kg = he_kg[core * BPC:(core + 1) * BPC]       # [16, 256, 3]
        qu = he_ques[core * BPC:(core + 1) * BPC]     # [16, 16, 3]
        uniq, inv = np.unique(np.concatenate([kg.ravel(), qu.ravel()]),
                              return_inverse=True)
        assert len(uniq) <= UMAX
        kg_c = inv[:kg.size].reshape(kg.shape)
        qu_c = inv[kg.size:].reshape(qu.shape)

        emb_c = np.zeros((UMAX, ES), bf)
        emb_c[0:len(uniq), 0:E] = emb[uniq].astype(bf)
        emb_c[0:len(uniq), E] = bf(1.0)

        # kg idx per pair: i = j*512 + (half*256 + s)
        kg_flat = np.zeros((8, NODES * 512), np.int64)
        for bp in range(8):
            blk = kg_c[2 * bp:2 * bp + 2]             # [2, 256, 3]
            kg_flat[bp] = blk.transpose(2, 0, 1).reshape(NODES, 512).reshape(-1)
        kg_idx = np.concatenate([wrap_idx(kg_flat[bp]) for bp in range(8)],
                                axis=1).astype(np.int16)

        # q idx: i = j*256 + (b*16 + q)
        q_flat = qu_c.transpose(2, 0, 1).reshape(-1)
        q_idx = wrap_idx(q_flat)

        m = dict(shared)
        m["emb"] = emb_c
        m["kg_idx"] = np.ascontiguousarray(kg_idx)
        m["q_idx"] = np.ascontiguousarray(q_idx)
        maps.append(m)
    return maps


def kernel(**inputs):
    global _CACHED
    if _CACHED is None:
        _CACHED = _build()
    nc = _CACHED
    in_maps = make_in_maps(**inputs)
    res = run_bass_kernel_spmd(nc, in_maps, list(range(NCORES)))
    return np.concatenate([r["out"] for r in res.results], axis=0)


# revision 3
# speedup vs baseline: 3.1648x; 1.4386x over previous
"""HAN (hypergraph attention network) Trainium2 kernel, v2.

Data-parallel over batch: 8 cores x 16 batch elements, params replicated.
v2 pipeline: per-core vocabulary compaction (int16 idx) + bf16 padded
embedding table with a built-in ones column -> dma_gather(transpose=True)
lands activations k-major with zero PE transposes -> bf16 projections with
bias folded into the weight row for the ones column -> bilinear attention
(heads*queries = 128 partitions) -> softmax -> bf16 attention-value matmul
-> pooled -> fc -> candidate sim -> log_softmax.  fc/glove weights are
loaded as a few large resident tiles so the tail phases never stall on DMA.
"""

import numpy as np
import ml_dtypes
from contextlib import ExitStack

import concourse.bass as bass
import concourse.bacc as bacc
import concourse.tile as tile
from concourse import mybir
from concourse.bass_utils import run_bass_kernel_spmd

F32 = mybir.dt.float32
BF16 = mybir.dt.bfloat16
I16 = mybir.dt.int16
AF = mybir.ActivationFunctionType
ALU = mybir.AluOpType
AX = mybir.AxisListType

NCORES = 8
B = 128
BPC = B // NCORES          # 16 batch elems per core
NQ, NS, NODES = 16, 256, 3
V, E = 50000, 300
ES = 384                   # padded emb row (bf16) -> 768B, %256==0
UMAX = 13056               # per-core unique rows cap: 16*(256+16)*3 / ... hard bound
C, H, OUT, NA = 1024, 8, 300, 5000
CC = C // 128              # 8 c-chunks
NCH = 9                    # (node j, k-chunk c) pairs: 3x3
OCN = [128, 128, 44]       # OUT=300 -> 3 o-chunks
SIMCH = [512] * 9 + [392]  # NA=5000 N-chunks
FCT = 4                    # fcw resident tiles
FCC = H * CC // FCT        # (h,cc) chunks per fcw tile

_CACHED = None


def _emit(ctx, tc, ins, outs):
    nc = tc.nc

    emb = ins["emb"]            # [UMAX, ES] bf16
    kg_idx = ins["kg_idx"]      # [128, 8*96] i16 (pair-major)
    q_idx = ins["q_idx"]        # [128, 48] i16
    kwT_d = ins["kwT"]          # [128, NCH*1024] bf16
    qwT_d = ins["qwT"]
    h2aT_d = ins["h2aT"]        # [128, CC*H] f32
    fcb_d = ins["fcb"]          # [128, 3] f32
    sel1_d = ins["sel1"]        # [128, H] f32
    sel2_d = ins["sel2"]        # [H, 128] f32
    idbf_d = ins["idbf"]        # [128, 128] bf16
    fcwT_d = ins["fcwT"]        # [128, H*CC*OUT] bf16
    gloT_d = ins["gloT"]        # [3, 128, NA] bf16
    out_d = outs["out"]         # [BPC, NA] f32

    const = ctx.enter_context(tc.tile_pool(name="const", bufs=1))
    actp = ctx.enter_context(tc.tile_pool(name="actp", bufs=2))
    hstp = ctx.enter_context(tc.tile_pool(name="hstp", bufs=2))
    hsbp = ctx.enter_context(tc.tile_pool(name="hsbp", bufs=2))
    xtp = ctx.enter_context(tc.tile_pool(name="xtp", bufs=2))
    attp = ctx.enter_context(tc.tile_pool(name="attp", bufs=2))
    tmpp = ctx.enter_context(tc.tile_pool(name="tmpp", bufs=2))
    smlp = ctx.enter_context(tc.tile_pool(name="smlp", bufs=2))

    pstr = ctx.enter_context(tc.tile_pool(name="pstr", bufs=3, space="PSUM"))
    pspj = ctx.enter_context(tc.tile_pool(name="pspj", bufs=3, space="PSUM"))
    psyt = ctx.enter_context(tc.tile_pool(name="psyt", bufs=2, space="PSUM"))

    # ---- resident constants / weights ----
    itq = const.tile([128, 48], I16, tag="itq")
    nc.sync.dma_start(itq[:], q_idx[:])
    itk = const.tile([128, 8 * 96], I16, tag="itk")
    nc.sync.dma_start(itk[:], kg_idx[:])
    qwT = const.tile([128, NCH * 1024], BF16, tag="qwT")
    nc.sync.dma_start(qwT[:], qwT_d[:])
    kwT = const.tile([128, NCH * 1024], BF16, tag="kwT")
    nc.sync.dma_start(kwT[:], kwT_d[:])
    h2aT = const.tile([128, CC * H], F32, tag="h2aT")
    nc.sync.dma_start(h2aT[:], h2aT_d[:])
    fcb = const.tile([128, 3], F32, tag="fcb")
    nc.sync.dma_start(fcb[:], fcb_d[:])
    sel1 = const.tile([128, H], F32, tag="sel1")
    nc.sync.dma_start(sel1[:], sel1_d[:])
    sel2 = const.tile([H, 128], F32, tag="sel2")
    nc.sync.dma_start(sel2[:], sel2_d[:])
    idbf = const.tile([128, 128], BF16, tag="idbf")
    nc.sync.dma_start(idbf[:], idbf_d[:])
    fcw_sb = []
    for t in range(FCT):
        fw = const.tile([128, FCC * OUT], BF16, tag=f"fcw{t}")
        nc.sync.dma_start(fw[:], fcwT_d[:, t * FCC * OUT:(t + 1) * FCC * OUT])
        fcw_sb.append(fw)
    glo_sb = []
    for oc in range(3):
        gs = const.tile([128, NA], BF16, tag=f"glo{oc}")
        nc.sync.dma_start(gs[:], gloT_d[oc])
        glo_sb.append(gs)

    hqT = const.tile([128, CC * 256], F32, tag="hqT")      # [c, b*16+q]
    POOL = const.tile([128, CC * BPC * H], F32, tag="POOL")  # col cc*128+b*8+h
    POOLb = const.tile([128, CC * BPC * H], BF16, tag="POOLb")
    fcout = const.tile([128, 3 * BPC], BF16, tag="fcout")
    sim_sb = const.tile([BPC, NA], F32, tag="sim_sb")
    parti = const.tile([BPC, 16], F32, tag="parti")
    lse = const.tile([BPC, 1], F32, tag="lse")
    tot = const.tile([BPC, 1], F32, tag="tot")

    def gather(idx_slice, ntok):
        """ntok tokens x NODES rows, j-major idx order; num_idxs > 768
        crashes the gather ucode, so one 768-idx gather per contiguous
        [3, 768] block. Returns list of [128, 3, 768] views."""
        ni = NODES * ntok
        ng = ni // 768
        at = actp.tile([128, 3 * ni], BF16, tag="act")
        views = []
        for g in range(ng):
            v = at[:, g * 2304:(g + 1) * 2304].rearrange(
                "p (c n) -> p c n", c=3)
            nc.gpsimd.dma_gather(
                out_ap=v,
                in_ap=emb[:],
                idxs_ap=idx_slice[:, g * 48:(g + 1) * 48],
                num_idxs=768,
                num_idxs_reg=768,
                elem_size=ES,
                transpose=True,
            )
            views.append(v)
        return views

    def project(wT, atvs, dstT, ntok):
        """dstT[:, cc*ntok : +ntok] = wT.T @ act (+bias via ones column).

        rhs pieces per (node j, k-chunk c): with one gather (ntok=256) the
        j-blocks are whole; with two (ntok=512) node 1 straddles the two
        gather blocks, so it contributes two half-range matmuls."""
        if len(atvs) == 1:
            pieces = [(j * 3 + c, atvs[0][:, c, j * ntok:(j + 1) * ntok], 0, ntok)
                      for j in range(3) for c in range(3)]
        else:
            pieces = (
                [(c, atvs[0][:, c, 0:512], 0, 512) for c in range(3)]
                + [(3 + c, atvs[0][:, c, 512:768], 0, 256) for c in range(3)]
                + [(3 + c, atvs[1][:, c, 0:256], 256, 512) for c in range(3)]
                + [(6 + c, atvs[1][:, c, 256:768], 0, 512) for c in range(3)]
            )
            # full-range ops must open and close the accumulation group
            pieces = pieces[0:3] + pieces[3:9] + pieces[9:12]
        for cc in range(CC):
            ps = pspj.tile([128, 512], F32, tag="pjps")
            for i, (ch, rhs, a, bnd) in enumerate(pieces):
                nc.tensor.matmul(
                    out=ps[:, a:bnd],
                    lhsT=wT[:, ch * 1024 + cc * 128: ch * 1024 + cc * 128 + 128],
                    rhs=rhs,
                    start=(i == 0),
                    stop=(i == len(pieces) - 1),
                )
            nc.scalar.copy(out=dstT[:, cc * ntok:(cc + 1) * ntok],
                           in_=ps[:, 0:ntok])

    # ---- prologue: hq for all 16 b (256 ques tokens) ----
    atq = gather(itq[:], 256)
    project(qwT, atq, hqT, 256)

    hqv = hqT[:].rearrange("p (c t) -> p c t", c=CC)  # [128, 8, 256]
    h2av = h2aT[:].rearrange("p (c h) -> p c h", c=CC)  # [128, 8, 8]
    pv = POOL[:].rearrange("p (c b h) -> p c b h", c=CC, b=BPC)

    # ---- per pair of batch elements ----
    for bp in range(BPC // 2):
        atk = gather(itk[:, bp * 96:(bp + 1) * 96], 512)

        hsT = hstp.tile([128, CC * 512], BF16, tag="hsT")
        project(kwT, atk, hsT, 512)

        for half in range(2):
            b = bp * 2 + half
            hb = half * 256

            # hs token-major bf16: [s-chunk partitions, col st*1024 + c]
            hs_sb = hsbp.tile([128, 2 * 1024], BF16, tag="hs_sb")
            for st in range(2):
                ps = pstr.tile([128, 1024], BF16, tag="trps")
                for cc in range(CC):
                    nc.tensor.transpose(
                        out=ps[:, cc * 128:(cc + 1) * 128],
                        in_=hsT[:, cc * 512 + hb + st * 128: cc * 512 + hb + st * 128 + 128],
                        identity=idbf[:],
                    )
                nc.scalar.copy(out=hs_sb[:, st * 1024:(st + 1) * 1024], in_=ps[:])

            # X^T[c, h*16+q] = hqT[c, q] * h2aT[c, h]
            XT = xtp.tile([128, 1024], BF16, tag="XT")
            nc.vector.tensor_tensor(
                out=XT[:].rearrange("p (c h q) -> p c h q", c=CC, h=H),
                in0=hqv[:, :, b * 16: b * 16 + 16].unsqueeze(2).to_broadcast(
                    [128, CC, H, 16]),
                in1=h2av[:, :, :].unsqueeze(3).to_broadcast([128, CC, H, 16]),
                op=ALU.mult,
            )

            # logits[hq=128, s=256]
            plg = pspj.tile([128, 512], F32, tag="pjps")
            for cc in range(CC):
                nc.tensor.matmul(
                    out=plg[:, 0:256],
                    lhsT=XT[:, cc * 128: cc * 128 + 128],
                    rhs=hsT[:, cc * 512 + hb: cc * 512 + hb + 256],
                    start=(cc == 0),
                    stop=(cc == CC - 1),
                )

            # softmax over flat (q,s) per (b,h); logits tiny -> skip max-sub
            att = attp.tile([128, 256], BF16, tag="att")
            qsum = smlp.tile([128, 1], F32, tag="qsum")
            nc.scalar.activation(att[:], plg[:, 0:256], AF.Exp, accum_out=qsum[:])

            dps = psyt.tile([128, 512], F32, tag="ytps", name="dps")
            nc.tensor.matmul(out=dps[0:8, 0:1], lhsT=sel1[:], rhs=qsum[:],
                             start=True, stop=True)
            r8 = smlp.tile([8, 1], F32, tag="r8")
            nc.vector.reciprocal(r8[:], dps[0:8, 0:1])
            nc.tensor.matmul(out=dps[:, 1:2], lhsT=sel2[:], rhs=r8[:],
                             start=True, stop=True)
            rsb = smlp.tile([128, 1], F32, tag="rsb")
            nc.vector.tensor_copy(rsb[:], dps[:, 1:2])

            attn = attp.tile([128, 256], BF16, tag="attn")
            nc.vector.tensor_scalar_mul(attn[:], att[:], rsb[:])

            # attT [s, hq] bf16
            attT = attp.tile([128, 256], BF16, tag="attT")
            psTb = pstr.tile([128, 256], BF16, tag="trps", name="psTb")
            for st in range(2):
                nc.tensor.transpose(
                    out=psTb[:, st * 128:(st + 1) * 128],
                    in_=attn[:, st * 128:(st + 1) * 128],
                    identity=idbf[:],
                )
            nc.vector.tensor_copy(attT[:], psTb[:])

            # YT[c, hq] per c-chunk; pooled[h,c] = sum_q hqT * sum_s attT*hs
            for ccg in range(2):
                py = psyt.tile([128, 512], F32, tag="ytps")
                for i in range(4):
                    cc = ccg * 4 + i
                    for st in range(2):
                        nc.tensor.matmul(
                            out=py[:, i * 128:(i + 1) * 128],
                            lhsT=hs_sb[:, st * 1024 + cc * 128: st * 1024 + cc * 128 + 128],
                            rhs=attT[:, st * 128:(st + 1) * 128],
                            start=(st == 0),
                            stop=(st == 1),
                        )
                tmp = tmpp.tile([128, 512], F32, tag="tmp")
                nc.vector.tensor_tensor(
                    out=tmp[:].rearrange("p (c h q) -> p c h q", c=4, h=H),
                    in0=py[:].rearrange("p (c h q) -> p c h q", c=4, h=H),
                    in1=hqv[:, ccg * 4:(ccg + 1) * 4, b * 16: b * 16 + 16].unsqueeze(2).to_broadcast([128, 4, H, 16]),
                    op=ALU.mult,
                )
                nc.vector.reduce_sum(
                    out=pv[:, ccg * 4:(ccg + 1) * 4, b, :],
                    in_=tmp[:].rearrange("p (c h q) -> p c h q", c=4, h=H),
                    axis=AX.X,
                )

    # ---- fc: out[o, b] = sum_{h,c} fc_w[o, h*1024+c] * pooled ----
    nc.vector.tensor_copy(POOLb[:], POOL[:])
    poolv = POOLb[:].rearrange("p (c b h) -> p c b h", c=CC, b=BPC)
    pfc = [pspj.tile([128, 512], F32, tag="pjps", name="pfc0"),
           pstr.tile([128, 512], F32, tag="trps", name="pfc1"),
           psyt.tile([128, 512], F32, tag="ytps", name="pfc2")]
    nhc = H * CC
    for h in range(H):
        for cc in range(CC):
            i = h * CC + cc
            fw = fcw_sb[i // FCC]
            fo = (i % FCC) * OUT
            for oc in range(3):
                ocn = OCN[oc]
                nc.tensor.matmul(
                    out=pfc[oc][0:ocn, 0:16],
                    lhsT=fw[:, fo + oc * 128: fo + oc * 128 + ocn],
                    rhs=poolv[:, cc, :, h],
                    start=(i == 0),
                    stop=(i == nhc - 1),
                )
    for oc in range(3):
        ocn = OCN[oc]
        nc.scalar.activation(
            out=fcout[0:ocn, oc * 16: oc * 16 + 16],
            in_=pfc[oc][0:ocn, 0:16],
            func=AF.Identity,
            bias=fcb[0:ocn, oc: oc + 1],
        )

    # ---- sim = fcout.T @ gloveT ; log_softmax over NA ----
    a0 = 0
    for ci, n in enumerate(SIMCH):
        pss = psyt.tile([16, 512], F32, tag="ytps", name="pss")
        for oc in range(3):
            ocn = OCN[oc]
            nc.tensor.matmul(
                out=pss[0:16, 0:n],
                lhsT=fcout[0:ocn, oc * 16: oc * 16 + 16],
                rhs=glo_sb[oc][0:ocn, a0: a0 + n],
                start=(oc == 0),
                stop=(oc == 2),
            )
        junk = tmpp.tile([128, 512], F32, tag="tmp")
        nc.scalar.activation(junk[0:16, 0:n], pss[0:16, 0:n], AF.Exp,
                             accum_out=parti[:, ci: ci + 1])
        nc.vector.tensor_copy(sim_sb[:, a0: a0 + n], pss[0:16, 0:n])
        a0 += n

    nc.vector.reduce_sum(out=tot[:], in_=parti[:, 0:10], axis=AX.X)
    nc.scalar.activation(lse[:], tot[:], AF.Ln)
    # chunked subtract + store so the output DMA overlaps the DVE work
    a0 = 0
    for n in SIMCH:
        nc.vector.tensor_scalar_sub(sim_sb[:, a0:a0 + n],
                                    sim_sb[:, a0:a0 + n], lse[:])
        nc.sync.dma_start(out_d[:, a0:a0 + n], sim_sb[:, a0:a0 + n])
        a0 += n


def _build():
    nc = bacc.Bacc("TRN2", target_bir_lowering=False, debug=False,
                   num_devices=NCORES)
    ins = {}

    def di(name, shape, dtype):
        ins[name] = nc.dram_tensor(name, list(shape), dtype,
                                   kind="ExternalInput").ap()

    di("emb", (UMAX, ES), BF16)
    di("kg_idx", (128, 8 * 96), I16)
    di("q_idx", (128, 48), I16)
    di("kwT", (128, NCH * 1024), BF16)
    di("qwT", (128, NCH * 1024), BF16)
    di("h2aT", (128, CC * H), F32)
    di("fcb", (128, 3), F32)
    di("sel1", (128, H), F32)
    di("sel2", (H, 128), F32)
    di("idbf", (128, 128), BF16)
    di("fcwT", (128, H * CC * OUT), BF16)
    di("gloT", (3, 128, NA), BF16)
    outs = {"out": nc.dram_tensor("out", [BPC, NA], F32,
                                  kind="ExternalOutput").ap()}

    with tile.TileContext(nc) as tc, ExitStack() as ctx:
        _emit(ctx, tc, ins, outs)
    nc.compile()
    return nc


def _pack_wT(W, bias):
    """[C, 900] f32 -> [128, NCH*1024] bf16 with bias folded at (0,2) row 44."""
    bf = ml_dtypes.bfloat16
    P = np.zeros((128, NCH * 1024), np.float32)
    for j in range(NODES):
        for c in range(3):
            ch = j * 3 + c
            kr = min(128, E - c * 128)
            P[0:kr, ch * 1024:(ch + 1) * 1024] = \
                W[:, j * E + c * 128: j * E + c * 128 + kr].T
    P[44, 2 * 1024:3 * 1024] = bias      # chunk (0,2) row 44 <- ones column
    return P.astype(bf)


def make_in_maps(he_ques, he_kg, emb, q2h_w, q2h_b, k2h_w, k2h_b,
                 h2att_w, h2att_b, fc_w, fc_b, glove_cands):
    f32 = np.float32
    bf = ml_dtypes.bfloat16
    emb = np.asarray(emb, f32)
    he_kg = np.asarray(he_kg).astype(np.int64)
    he_ques = np.asarray(he_ques).astype(np.int64)

    kwT = _pack_wT(np.asarray(k2h_w, f32), np.asarray(k2h_b, f32))
    qwT = _pack_wT(np.asarray(q2h_w, f32), np.asarray(q2h_b, f32))

    h2aT = np.zeros((128, CC * H), f32)
    for cc in range(CC):
        h2aT[:, cc * H:(cc + 1) * H] = np.asarray(h2att_w, f32)[:, cc * 128:(cc + 1) * 128].T

    fcb = np.zeros((128, 3), f32)
    fcb_src = np.asarray(fc_b, f32)
    for oc in range(3):
        fcb[0:OCN[oc], oc] = fcb_src[oc * 128: oc * 128 + OCN[oc]]

    sel1 = np.zeros((128, H), f32)
    for p in range(128):
        sel1[p, p // 16] = 1.0
    sel2 = np.ascontiguousarray(sel1.T)
    idbf = np.eye(128, dtype=bf)

    fcw = np.asarray(fc_w, f32).reshape(OUT, H, CC, 128)
    fcwT = np.ascontiguousarray(
        fcw.transpose(3, 1, 2, 0).reshape(128, H * CC * OUT)).astype(bf)

    glo = np.asarray(glove_cands, f32)
    gloT = np.zeros((3, 128, NA), f32)
    for oc in range(3):
        gloT[oc, 0:OCN[oc], :] = glo[:, oc * 128: oc * 128 + OCN[oc]].T
    gloT = gloT.astype(bf)

    shared = dict(kwT=kwT, qwT=qwT, h2aT=h2aT, fcb=fcb, sel1=sel1,
                  sel2=sel2, idbf=idbf, fcwT=fcwT, gloT=gloT)

    def wrap_idx(flat):
        """[n] -> [128, n//16] int16 wrapped in 16 partitions, replicated."""
        n = flat.shape[0]
        t = np.zeros((128, n // 16), np.int16)
        t[0:16] = flat.reshape(n // 16, 16).T
        for g in range(1, 8):
            t[g * 16:(g + 1) * 16] = t[0:16]
        return t

    maps = []
    for core in range(NCORES):
        kg = he_kg[core * BPC:(core + 1) * BPC]       # [16, 256, 3]
        qu = he_ques[core * BPC:(core + 1) * BPC]     # [16, 16, 3]
        uniq, inv = np.unique(np.concatenate([kg.ravel(), qu.ravel()]),
                              return_inverse=True)
        assert len(uniq) <= UMAX
        kg_c = inv[:kg.size].reshape(kg.shape)
        qu_c = inv[kg.size:].reshape(qu.shape)

        emb_c = np.zeros((UMAX, ES), bf)
        emb_c[0:len(uniq), 0:E] = emb[uniq].astype(bf)
        emb_c[0:len(uniq), E] = bf(1.0)

        # kg idx per pair: i = j*512 + (half*256 + s)
        kg_flat = np.zeros((8, NODES * 512), np.int64)
        for bp in range(8):
            blk = kg_c[2 * bp:2 * bp + 2]             # [2, 256, 3]
            kg_flat[bp] = blk.transpose(2, 0, 1).reshape(NODES, 512).reshape(-1)
        kg_idx = np.concatenate([wrap_idx(kg_flat[bp]) for bp in range(8)],
                                axis=1).astype(np.int16)

        # q idx: i = j*256 + (b*16 + q)
        q_flat = qu_c.transpose(2, 0, 1).reshape(-1)
        q_idx = wrap_idx(q_flat)

        m = dict(shared)
        m["emb"] = emb_c
        m["kg_idx"] = np.ascontiguousarray(kg_idx)
        m["q_idx"] = np.ascontiguousarray(q_idx)
        maps.append(m)
    return maps


def kernel(**inputs):
    global _CACHED
    if _CACHED is None:
        _CACHED = _build()
    nc = _CACHED
    in_maps = make_in_maps(**inputs)
    res = run_bass_kernel_spmd(nc, in_maps, list(range(NCORES)))
    return np.concatenate([r["out"] for r in res.results], axis=0)


# revision 8
# speedup vs baseline: 4.0279x; 1.2727x over previous
"""HAN (hypergraph attention network) Trainium2 kernel, v2.

Data-parallel over batch: 8 cores x 16 batch elements, params replicated.
v2 pipeline: per-core vocabulary compaction (int16 idx) + bf16 padded
embedding table with a built-in ones column -> dma_gather(transpose=True)
lands activations k-major with zero PE transposes -> bf16 projections with
bias folded into the weight row for the ones column -> bilinear attention
(heads*queries = 128 partitions) -> softmax -> bf16 attention-value matmul
-> pooled -> fc -> candidate sim -> log_softmax.  fc/glove weights are
loaded as a few large resident tiles so the tail phases never stall on DMA.
"""

import numpy as np
import ml_dtypes
from contextlib import ExitStack

import concourse.bass as bass
import concourse.bacc as bacc
import concourse.tile as tile
from concourse import mybir
from concourse.bass_utils import run_bass_kernel_spmd

F32 = mybir.dt.float32
BF16 = mybir.dt.bfloat16
I16 = mybir.dt.int16
AF = mybir.ActivationFunctionType
ALU = mybir.AluOpType
AX = mybir.AxisListType

NCORES = 8
B = 128
BPC = B // NCORES          # 16 batch elems per core
NQ, NS, NODES = 16, 256, 3
V, E = 50000, 300
ES = 384                   # padded emb row (bf16) -> 768B, %256==0
UMAX = 13056               # per-core unique rows cap: 16*(256+16)*3 / ... hard bound
C, H, OUT, NA = 1024, 8, 300, 5000
CC = C // 128              # 8 c-chunks
NCH = 9                    # (node j, k-chunk c) pairs: 3x3
OCN = [128, 128, 44]       # OUT=300 -> 3 o-chunks
SIMCH = [512] * 9 + [392]  # NA=5000 N-chunks
FCT = 4                    # fcw resident tiles
FCC = H * CC // FCT        # (h,cc) chunks per fcw tile

_CACHED = None


def _emit(ctx, tc, ins, outs):
    nc = tc.nc

    # merged inputs (fewer per-call args = less dispatch overhead):
    # wb bf16: qwT | kwT | idbf | fcwT | glo0..2 ; wf f32: h2aT | fcb |
    # sel1 | sel2 ; wi i16: q_idx | kg_idx
    emb = ins["emb"]            # [UMAX, ES] bf16
    wb = ins["wb"]              # [128, WB] bf16
    wf = ins["wf"]              # [128, WF] f32
    wi = ins["wi"]              # [128, WI] i16
    out_d = outs["out"]         # [BPC, NA] f32

    const = ctx.enter_context(tc.tile_pool(name="const", bufs=1))
    actp = ctx.enter_context(tc.tile_pool(name="actp", bufs=2))
    hstp = ctx.enter_context(tc.tile_pool(name="hstp", bufs=2))
    hsbp = ctx.enter_context(tc.tile_pool(name="hsbp", bufs=2))
    xtp = ctx.enter_context(tc.tile_pool(name="xtp", bufs=2))
    attp = ctx.enter_context(tc.tile_pool(name="attp", bufs=2))
    tmpp = ctx.enter_context(tc.tile_pool(name="tmpp", bufs=2))
    smlp = ctx.enter_context(tc.tile_pool(name="smlp", bufs=2))

    pstr = ctx.enter_context(tc.tile_pool(name="pstr", bufs=3, space="PSUM"))
    pspj = ctx.enter_context(tc.tile_pool(name="pspj", bufs=3, space="PSUM"))
    psyt = ctx.enter_context(tc.tile_pool(name="psyt", bufs=2, space="PSUM"))

    # ---- resident constants / weights (sliced out of the merged bufs) ----
    itq = const.tile([128, 48], I16, tag="itq")
    nc.sync.dma_start(itq[:], wi[:, 0:48])
    itk = const.tile([128, 8 * 96], I16, tag="itk")
    nc.sync.dma_start(itk[:], wi[:, 48:816])
    qwT = const.tile([128, NCH * 1024], BF16, tag="qwT")
    nc.sync.dma_start(qwT[:], wb[:, 0:9216])
    kwT = const.tile([128, NCH * 1024], BF16, tag="kwT")
    nc.sync.dma_start(kwT[:], wb[:, 9216:18432])
    h2aT = const.tile([128, CC * H], F32, tag="h2aT")
    nc.sync.dma_start(h2aT[:], wf[:, 0:64])
    fcb = const.tile([128, 3], F32, tag="fcb")
    nc.sync.dma_start(fcb[:], wf[:, 64:67])
    sel1 = const.tile([128, H], F32, tag="sel1")
    nc.sync.dma_start(sel1[:], wf[:, 67:75])
    sel2 = const.tile([H, 128], F32, tag="sel2")
    nc.sync.dma_start(sel2[:], wf[0:H, 75:203])
    idbf = const.tile([128, 128], BF16, tag="idbf")
    nc.sync.dma_start(idbf[:], wb[:, 18432:18560])
    fcw_sb = []
    for t in range(FCT):
        fw = const.tile([128, FCC * OUT], BF16, tag=f"fcw{t}")
        nc.sync.dma_start(
            fw[:], wb[:, 18560 + t * FCC * OUT: 18560 + (t + 1) * FCC * OUT])
        fcw_sb.append(fw)
    glo_sb = []
    for oc in range(3):
        gs = const.tile([128, NA], BF16, tag=f"glo{oc}")
        nc.sync.dma_start(gs[:], wb[:, 37760 + oc * NA: 37760 + (oc + 1) * NA])
        glo_sb.append(gs)

    hqT = const.tile([128, CC * 256], F32, tag="hqT")      # [c, b*16+q]
    POOL = const.tile([128, CC * BPC * H], F32, tag="POOL")  # col cc*128+b*8+h
    POOLb = const.tile([128, CC * BPC * H], BF16, tag="POOLb")
    fcout = const.tile([128, 3 * BPC], BF16, tag="fcout")
    sim_sb = const.tile([BPC, NA], F32, tag="sim_sb")
    parti = const.tile([BPC, 16], F32, tag="parti")
    lse = const.tile([BPC, 1], F32, tag="lse")
    tot = const.tile([BPC, 1], F32, tag="tot")

    def gather(idx_slice, ntok):
        """ntok tokens x NODES rows, j-major idx order; num_idxs > 768
        crashes the gather ucode, so one 768-idx gather per contiguous
        [3, 768] block. Returns list of [128, 3, 768] views."""
        ni = NODES * ntok
        ng = ni // 768
        at = actp.tile([128, 3 * ni], BF16, tag="act")
        views = []
        for g in range(ng):
            v = at[:, g * 2304:(g + 1) * 2304].rearrange(
                "p (c n) -> p c n", c=3)
            nc.gpsimd.dma_gather(
                out_ap=v,
                in_ap=emb[:],
                idxs_ap=idx_slice[:, g * 48:(g + 1) * 48],
                num_idxs=768,
                num_idxs_reg=768,
                elem_size=ES,
                transpose=True,
            )
            views.append(v)
        return views

    def project(wT, atvs, dstT, ntok):
        """dstT[:, cc*ntok : +ntok] = wT.T @ act (+bias via ones column).

        rhs pieces per (node j, k-chunk c): with one gather (ntok=256) the
        j-blocks are whole; with two (ntok=512) node 1 straddles the two
        gather blocks, so it contributes two half-range matmuls."""
        if len(atvs) == 1:
            pieces = [(j * 3 + c, atvs[0][:, c, j * ntok:(j + 1) * ntok], 0, ntok)
                      for j in range(3) for c in range(3)]
        else:
            pieces = (
                [(c, atvs[0][:, c, 0:512], 0, 512) for c in range(3)]
                + [(3 + c, atvs[0][:, c, 512:768], 0, 256) for c in range(3)]
                + [(3 + c, atvs[1][:, c, 0:256], 256, 512) for c in range(3)]
                + [(6 + c, atvs[1][:, c, 256:768], 0, 512) for c in range(3)]
            )
            # full-range ops must open and close the accumulation group
            pieces = pieces[0:3] + pieces[3:9] + pieces[9:12]
        for cc in range(CC):
            ps = pspj.tile([128, 512], F32, tag="pjps")
            for i, (ch, rhs, a, bnd) in enumerate(pieces):
                nc.tensor.matmul(
                    out=ps[:, a:bnd],
                    lhsT=wT[:, ch * 1024 + cc * 128: ch * 1024 + cc * 128 + 128],
                    rhs=rhs,
                    start=(i == 0),
                    stop=(i == len(pieces) - 1),
                )
            nc.scalar.copy(out=dstT[:, cc * ntok:(cc + 1) * ntok],
                           in_=ps[:, 0:ntok])

    # ---- prologue: hq for all 16 b (256 ques tokens) ----
    atq = gather(itq[:], 256)
    project(qwT, atq, hqT, 256)

    hqv = hqT[:].rearrange("p (c t) -> p c t", c=CC)  # [128, 8, 256]
    h2av = h2aT[:].rearrange("p (c h) -> p c h", c=CC)  # [128, 8, 8]
    pv = POOL[:].rearrange("p (c b h) -> p c b h", c=CC, b=BPC)

    # ---- per pair of batch elements ----
    for bp in range(BPC // 2):
        atk = gather(itk[:, bp * 96:(bp + 1) * 96], 512)

        hsT = hstp.tile([128, CC * 512], BF16, tag="hsT")
        project(kwT, atk, hsT, 512)

        for half in range(2):
            b = bp * 2 + half
            hb = half * 256

            # hs token-major bf16: [s-chunk partitions, col st*1024 + c]
            hs_sb = hsbp.tile([128, 2 * 1024], BF16, tag="hs_sb")
            for st in range(2):
                ps = pstr.tile([128, 1024], BF16, tag="trps")
                for cc in range(CC):
                    nc.tensor.transpose(
                        out=ps[:, cc * 128:(cc + 1) * 128],
                        in_=hsT[:, cc * 512 + hb + st * 128: cc * 512 + hb + st * 128 + 128],
                        identity=idbf[:],
                    )
                nc.scalar.copy(out=hs_sb[:, st * 1024:(st + 1) * 1024], in_=ps[:])

            # X^T[c, h*16+q] = hqT[c, q] * h2aT[c, h]
            XT = xtp.tile([128, 1024], BF16, tag="XT")
            nc.vector.tensor_tensor(
                out=XT[:].rearrange("p (c h q) -> p c h q", c=CC, h=H),
                in0=hqv[:, :, b * 16: b * 16 + 16].unsqueeze(2).to_broadcast(
                    [128, CC, H, 16]),
                in1=h2av[:, :, :].unsqueeze(3).to_broadcast([128, CC, H, 16]),
                op=ALU.mult,
            )

            # logits[hq=128, s=256]
            plg = pspj.tile([128, 512], F32, tag="pjps")
            for cc in range(CC):
                nc.tensor.matmul(
                    out=plg[:, 0:256],
                    lhsT=XT[:, cc * 128: cc * 128 + 128],
                    rhs=hsT[:, cc * 512 + hb: cc * 512 + hb + 256],
                    start=(cc == 0),
                    stop=(cc == CC - 1),
                )

            # softmax over flat (q,s) per (b,h); logits tiny -> skip max-sub
            att = attp.tile([128, 256], BF16, tag="att")
            qsum = smlp.tile([128, 1], F32, tag="qsum")
            nc.scalar.activation(att[:], plg[:, 0:256], AF.Exp, accum_out=qsum[:])

            dps = psyt.tile([128, 512], F32, tag="ytps", name="dps")
            nc.tensor.matmul(out=dps[0:8, 0:1], lhsT=sel1[:], rhs=qsum[:],
                             start=True, stop=True)
            r8 = smlp.tile([8, 1], F32, tag="r8")
            nc.vector.reciprocal(r8[:], dps[0:8, 0:1])
            nc.tensor.matmul(out=dps[:, 1:2], lhsT=sel2[:], rhs=r8[:],
                             start=True, stop=True)
            rsb = smlp.tile([128, 1], F32, tag="rsb")
            nc.vector.tensor_copy(rsb[:], dps[:, 1:2])

            attn = attp.tile([128, 256], BF16, tag="attn")
            nc.vector.tensor_scalar_mul(attn[:], att[:], rsb[:])

            # attT [s, hq] bf16
            attT = attp.tile([128, 256], BF16, tag="attT")
            psTb = pstr.tile([128, 256], BF16, tag="trps", name="psTb")
            for st in range(2):
                nc.tensor.transpose(
                    out=psTb[:, st * 128:(st + 1) * 128],
                    in_=attn[:, st * 128:(st + 1) * 128],
                    identity=idbf[:],
                )
            nc.vector.tensor_copy(attT[:], psTb[:])

            # YT[c, hq] per c-chunk; pooled[h,c] = sum_q hqT * sum_s attT*hs
            for ccg in range(2):
                py = psyt.tile([128, 512], F32, tag="ytps")
                for i in range(4):
                    cc = ccg * 4 + i
                    for st in range(2):
                        nc.tensor.matmul(
                            out=py[:, i * 128:(i + 1) * 128],
                            lhsT=hs_sb[:, st * 1024 + cc * 128: st * 1024 + cc * 128 + 128],
                            rhs=attT[:, st * 128:(st + 1) * 128],
                            start=(st == 0),
                            stop=(st == 1),
                        )
                tmp = tmpp.tile([128, 512], F32, tag="tmp")
                nc.vector.tensor_tensor(
                    out=tmp[:].rearrange("p (c h q) -> p c h q", c=4, h=H),
                    in0=py[:].rearrange("p (c h q) -> p c h q", c=4, h=H),
                    in1=hqv[:, ccg * 4:(ccg + 1) * 4, b * 16: b * 16 + 16].unsqueeze(2).to_broadcast([128, 4, H, 16]),
                    op=ALU.mult,
                )
                nc.vector.reduce_sum(
                    out=pv[:, ccg * 4:(ccg + 1) * 4, b, :],
                    in_=tmp[:].rearrange("p (c h q) -> p c h q", c=4, h=H),
                    axis=AX.X,
                )

    # ---- fc: out[o, b] = sum_{h,c} fc_w[o, h*1024+c] * pooled ----
    nc.vector.tensor_copy(POOLb[:], POOL[:])
    poolv = POOLb[:].rearrange("p (c b h) -> p c b h", c=CC, b=BPC)
    pfc = [pspj.tile([128, 512], F32, tag="pjps", name="pfc0"),
           pstr.tile([128, 512], F32, tag="trps", name="pfc1"),
           psyt.tile([128, 512], F32, tag="ytps", name="pfc2")]
    nhc = H * CC
    for h in range(H):
        for cc in range(CC):
            i = h * CC + cc
            fw = fcw_sb[i // FCC]
            fo = (i % FCC) * OUT
            for oc in range(3):
                ocn = OCN[oc]
                nc.tensor.matmul(
                    out=pfc[oc][0:ocn, 0:16],
                    lhsT=fw[:, fo + oc * 128: fo + oc * 128 + ocn],
                    rhs=poolv[:, cc, :, h],
                    start=(i == 0),
                    stop=(i == nhc - 1),
                )
    for oc in range(3):
        ocn = OCN[oc]
        nc.scalar.activation(
            out=fcout[0:ocn, oc * 16: oc * 16 + 16],
            in_=pfc[oc][0:ocn, 0:16],
            func=AF.Identity,
            bias=fcb[0:ocn, oc: oc + 1],
        )

    # ---- sim = fcout.T @ gloveT ; log_softmax over NA ----
    a0 = 0
    for ci, n in enumerate(SIMCH):
        pss = psyt.tile([16, 512], F32, tag="ytps", name="pss")
        for oc in range(3):
            ocn = OCN[oc]
            nc.tensor.matmul(
                out=pss[0:16, 0:n],
                lhsT=fcout[0:ocn, oc * 16: oc * 16 + 16],
                rhs=glo_sb[oc][0:ocn, a0: a0 + n],
                start=(oc == 0),
                stop=(oc == 2),
            )
        junk = tmpp.tile([128, 512], F32, tag="tmp")
        nc.scalar.activation(junk[0:16, 0:n], pss[0:16, 0:n], AF.Exp,
                             accum_out=parti[:, ci: ci + 1])
        nc.vector.tensor_copy(sim_sb[:, a0: a0 + n], pss[0:16, 0:n])
        a0 += n

    nc.vector.reduce_sum(out=tot[:], in_=parti[:, 0:10], axis=AX.X)
    nc.scalar.activation(lse[:], tot[:], AF.Ln)
    # chunked subtract + store so the output DMA overlaps the DVE work
    a0 = 0
    for n in SIMCH:
        nc.vector.tensor_scalar_sub(sim_sb[:, a0:a0 + n],
                                    sim_sb[:, a0:a0 + n], lse[:])
        nc.sync.dma_start(out_d[:, a0:a0 + n], sim_sb[:, a0:a0 + n])
        a0 += n


def _build():
    nc = bacc.Bacc("TRN2", target_bir_lowering=False, debug=False,
                   num_devices=NCORES)
    ins = {}

    def di(name, shape, dtype):
        ins[name] = nc.dram_tensor(name, list(shape), dtype,
                                   kind="ExternalInput").ap()

    di("emb", (UMAX, ES), BF16)
    di("wb", (128, 52760), BF16)
    di("wf", (128, 203), F32)
    di("wi", (128, 816), I16)
    outs = {"out": nc.dram_tensor("out", [BPC, NA], F32,
                                  kind="ExternalOutput").ap()}

    with tile.TileContext(nc) as tc, ExitStack() as ctx:
        _emit(ctx, tc, ins, outs)
    nc.compile()
    return nc


def _pack_wT(W, bias):
    """[C, 900] f32 -> [128, NCH*1024] bf16 with bias folded at (0,2) row 44."""
    bf = ml_dtypes.bfloat16
    P = np.zeros((128, NCH * 1024), np.float32)
    for j in range(NODES):
        for c in range(3):
            ch = j * 3 + c
            kr = min(128, E - c * 128)
            P[0:kr, ch * 1024:(ch + 1) * 1024] = \
                W[:, j * E + c * 128: j * E + c * 128 + kr].T
    P[44, 2 * 1024:3 * 1024] = bias      # chunk (0,2) row 44 <- ones column
    return P.astype(bf)


def make_in_maps(he_ques, he_kg, emb, q2h_w, q2h_b, k2h_w, k2h_b,
                 h2att_w, h2att_b, fc_w, fc_b, glove_cands):
    f32 = np.float32
    bf = ml_dtypes.bfloat16
    emb = np.asarray(emb, f32)
    he_kg = np.asarray(he_kg).astype(np.int64)
    he_ques = np.asarray(he_ques).astype(np.int64)

    kwT = _pack_wT(np.asarray(k2h_w, f32), np.asarray(k2h_b, f32))
    qwT = _pack_wT(np.asarray(q2h_w, f32), np.asarray(q2h_b, f32))

    h2aT = np.zeros((128, CC * H), f32)
    for cc in range(CC):
        h2aT[:, cc * H:(cc + 1) * H] = np.asarray(h2att_w, f32)[:, cc * 128:(cc + 1) * 128].T

    fcb = np.zeros((128, 3), f32)
    fcb_src = np.asarray(fc_b, f32)
    for oc in range(3):
        fcb[0:OCN[oc], oc] = fcb_src[oc * 128: oc * 128 + OCN[oc]]

    sel1 = np.zeros((128, H), f32)
    for p in range(128):
        sel1[p, p // 16] = 1.0
    sel2 = np.ascontiguousarray(sel1.T)
    idbf = np.eye(128, dtype=bf)

    fcw = np.asarray(fc_w, f32).reshape(OUT, H, CC, 128)
    fcwT = np.ascontiguousarray(
        fcw.transpose(3, 1, 2, 0).reshape(128, H * CC * OUT)).astype(bf)

    glo = np.asarray(glove_cands, f32)
    gloT = np.zeros((3, 128, NA), f32)
    for oc in range(3):
        gloT[oc, 0:OCN[oc], :] = glo[:, oc * 128: oc * 128 + OCN[oc]].T
    gloT = gloT.astype(bf)

    # merged bf16 buffer: qwT | kwT | idbf | fcwT | glo0..2
    wb = np.concatenate(
        [qwT, kwT, idbf, fcwT, gloT[0], gloT[1], gloT[2]], axis=1)
    # merged f32 buffer: h2aT | fcb | sel1 | sel2 (sel2 on partitions 0:H)
    wf = np.zeros((128, 203), f32)
    wf[:, 0:64] = h2aT
    wf[:, 64:67] = fcb
    wf[:, 67:75] = sel1
    wf[0:H, 75:203] = sel2
    shared = dict(wb=np.ascontiguousarray(wb), wf=wf)

    def wrap_idx(flat):
        """[n] -> [128, n//16] int16 wrapped in 16 partitions, replicated."""
        n = flat.shape[0]
        t = np.zeros((128, n // 16), np.int16)
        t[0:16] = flat.reshape(n // 16, 16).T
        for g in range(1, 8):
            t[g * 16:(g + 1) * 16] = t[0:16]
        return t

    maps = []
    for core in range(NCORES):
        kg = he_kg[core * BPC:(core + 1) * BPC]       # [16, 256, 3]
        qu = he_ques[core * BPC:(core + 1) * BPC]     # [16, 16, 3]
        uniq, inv = np.unique(np.concatenate([kg.ravel(), qu.ravel()]),
                              return_inverse=True)
        assert len(uniq) <= UMAX
        kg_c = inv[:kg.size].reshape(kg.shape)
        qu_c = inv[kg.size:].reshape(qu.shape)

        emb_c = np.zeros((UMAX, ES), bf)
        emb_c[0:len(uniq), 0:E] = emb[uniq].astype(bf)
        emb_c[0:len(uniq), E] = bf(1.0)

        # kg idx per pair: i = j*512 + (half*256 + s)
        kg_flat = np.zeros((8, NODES * 512), np.int64)
        for bp in range(8):
            blk = kg_c[2 * bp:2 * bp + 2]             # [2, 256, 3]
            kg_flat[bp] = blk.transpose(2, 0, 1).reshape(NODES, 512).reshape(-1)
        kg_idx = np.concatenate([wrap_idx(kg_flat[bp]) for bp in range(8)],
                                axis=1).astype(np.int16)

        # q idx: i = j*256 + (b*16 + q)
        q_flat = qu_c.transpose(2, 0, 1).reshape(-1)
        q_idx = wrap_idx(q_flat)

        wi = np.concatenate([q_idx, kg_idx], axis=1).astype(np.int16)

        m = dict(shared)
        m["emb"] = emb_c
        m["wi"] = np.ascontiguousarray(wi)
        maps.append(m)
    return maps


def kernel(**inputs):
    global _CACHED
    if _CACHED is None:
        _CACHED = _build()
    nc = _CACHED
    in_maps = make_in_maps(**inputs)
    res = run_bass_kernel_spmd(nc, in_maps, list(range(NCORES)))
    return np.concatenate([r["out"] for r in res.results], axis=0)


# revision 12
# speedup vs baseline: 4.1519x; 1.0308x over previous
"""HAN (hypergraph attention network) Trainium2 kernel, v2.

Data-parallel over batch: 8 cores x 16 batch elements, params replicated.
v2 pipeline: per-core vocabulary compaction (int16 idx) + bf16 padded
embedding table with a built-in ones column -> dma_gather(transpose=True)
lands activations k-major with zero PE transposes -> bf16 projections with
bias folded into the weight row for the ones column -> bilinear attention
(heads*queries = 128 partitions) -> softmax -> bf16 attention-value matmul
-> pooled -> fc -> candidate sim -> log_softmax.  fc/glove weights are
loaded as a few large resident tiles so the tail phases never stall on DMA.
"""

import numpy as np
import ml_dtypes
from contextlib import ExitStack

import concourse.bass as bass
import concourse.bacc as bacc
import concourse.tile as tile
from concourse import mybir
from concourse.bass_utils import run_bass_kernel_spmd

F32 = mybir.dt.float32
BF16 = mybir.dt.bfloat16
I16 = mybir.dt.int16
AF = mybir.ActivationFunctionType
ALU = mybir.AluOpType
AX = mybir.AxisListType

NCORES = 8
B = 128
BPC = B // NCORES          # 16 batch elems per core
NQ, NS, NODES = 16, 256, 3
V, E = 50000, 300
ES = 384                   # padded emb row (bf16) -> 768B, %256==0
UMAX = 13056               # per-core unique rows cap: 16*(256+16)*3 / ... hard bound
C, H, OUT, NA = 1024, 8, 300, 5000
CC = C // 128              # 8 c-chunks
NCH = 9                    # (node j, k-chunk c) pairs: 3x3
OCN = [128, 128, 44]       # OUT=300 -> 3 o-chunks
SIMCH = [512] * 9 + [392]  # NA=5000 N-chunks
FCT = 4                    # fcw resident tiles
FCC = H * CC // FCT        # (h,cc) chunks per fcw tile

_CACHED = None


def _emit(ctx, tc, ins, outs):
    nc = tc.nc

    # merged inputs (fewer per-call args = less dispatch overhead).
    # wb bf16 cols: 0:9216 qwT | 9216:18432 kwT | 18432:18560 idbf |
    # 18560:37760 fcwT | 37760:52760 glo | 52760:53166 f32 consts
    # (h2aT|fcb|sel1|sel2, bitcast) | 53166:53982 i16 idx (q|kg, bitcast)
    emb = ins["emb"]            # [UMAX, ES] bf16
    wb = ins["wb"]              # [128, 53982] bf16
    wf = wb[:, 52760:53166].bitcast(F32)   # [128, 203] f32
    wi = wb[:, 53166:53982].bitcast(I16)   # [128, 816] i16
    out_d = outs["out"]         # [BPC, NA] f32

    const = ctx.enter_context(tc.tile_pool(name="const", bufs=1))
    actp = ctx.enter_context(tc.tile_pool(name="actp", bufs=2))
    hstp = ctx.enter_context(tc.tile_pool(name="hstp", bufs=2))
    hsbp = ctx.enter_context(tc.tile_pool(name="hsbp", bufs=2))
    xtp = ctx.enter_context(tc.tile_pool(name="xtp", bufs=2))
    attp = ctx.enter_context(tc.tile_pool(name="attp", bufs=2))
    tmpp = ctx.enter_context(tc.tile_pool(name="tmpp", bufs=2))
    smlp = ctx.enter_context(tc.tile_pool(name="smlp", bufs=2))

    pstr = ctx.enter_context(tc.tile_pool(name="pstr", bufs=3, space="PSUM"))
    pspj = ctx.enter_context(tc.tile_pool(name="pspj", bufs=3, space="PSUM"))
    psyt = ctx.enter_context(tc.tile_pool(name="psyt", bufs=2, space="PSUM"))

    # ---- resident constants / weights (sliced out of the merged bufs) ----
    itq = const.tile([128, 48], I16, tag="itq")
    nc.sync.dma_start(itq[:], wi[:, 0:48])
    itk = const.tile([128, 8 * 96], I16, tag="itk")
    nc.sync.dma_start(itk[:], wi[:, 48:816])
    qwT = const.tile([128, NCH * 1024], BF16, tag="qwT")
    nc.sync.dma_start(qwT[:], wb[:, 0:9216])
    kwT = const.tile([128, NCH * 1024], BF16, tag="kwT")
    nc.sync.dma_start(kwT[:], wb[:, 9216:18432])
    h2aT = const.tile([128, CC * H], F32, tag="h2aT")
    nc.sync.dma_start(h2aT[:], wf[:, 0:64])
    fcb = const.tile([128, 3], F32, tag="fcb")
    nc.sync.dma_start(fcb[:], wf[:, 64:67])
    sel1 = const.tile([128, H], F32, tag="sel1")
    nc.sync.dma_start(sel1[:], wf[:, 67:75])
    sel2 = const.tile([H, 128], F32, tag="sel2")
    nc.sync.dma_start(sel2[:], wf[0:H, 75:203])
    idbf = const.tile([128, 128], BF16, tag="idbf")
    nc.sync.dma_start(idbf[:], wb[:, 18432:18560])
    fcw_sb = []
    for t in range(FCT):
        fw = const.tile([128, FCC * OUT], BF16, tag=f"fcw{t}")
        nc.sync.dma_start(
            fw[:], wb[:, 18560 + t * FCC * OUT: 18560 + (t + 1) * FCC * OUT])
        fcw_sb.append(fw)
    glo_sb = []
    for oc in range(3):
        gs = const.tile([128, NA], BF16, tag=f"glo{oc}")
        nc.sync.dma_start(gs[:], wb[:, 37760 + oc * NA: 37760 + (oc + 1) * NA])
        glo_sb.append(gs)

    hqT = const.tile([128, CC * 256], F32, tag="hqT")      # [c, b*16+q]
    POOL = const.tile([128, CC * BPC * H], F32, tag="POOL")  # col cc*128+b*8+h
    POOLb = const.tile([128, CC * BPC * H], BF16, tag="POOLb")
    fcout = const.tile([128, 3 * BPC], BF16, tag="fcout")
    sim_sb = const.tile([BPC, NA], F32, tag="sim_sb")
    parti = const.tile([BPC, 16], F32, tag="parti")
    lse = const.tile([BPC, 1], F32, tag="lse")
    tot = const.tile([BPC, 1], F32, tag="tot")

    def gather(idx_slice, ntok):
        """ntok tokens x NODES rows, j-major idx order; num_idxs > 768
        crashes the gather ucode, so one 768-idx gather per contiguous
        [3, 768] block. Returns list of [128, 3, 768] views."""
        ni = NODES * ntok
        ng = ni // 768
        at = actp.tile([128, 3 * ni], BF16, tag="act")
        views = []
        for g in range(ng):
            v = at[:, g * 2304:(g + 1) * 2304].rearrange(
                "p (c n) -> p c n", c=3)
            nc.gpsimd.dma_gather(
                out_ap=v,
                in_ap=emb[:],
                idxs_ap=idx_slice[:, g * 48:(g + 1) * 48],
                num_idxs=768,
                num_idxs_reg=768,
                elem_size=ES,
                transpose=True,
            )
            views.append(v)
        return views

    def project(wT, atvs, dstT, ntok):
        """dstT[:, cc*ntok : +ntok] = wT.T @ act (+bias via ones column).

        rhs pieces per (node j, k-chunk c): with one gather (ntok=256) the
        j-blocks are whole; with two (ntok=512) node 1 straddles the two
        gather blocks, so it contributes two half-range matmuls."""
        if len(atvs) == 1:
            pieces = [(j * 3 + c, atvs[0][:, c, j * ntok:(j + 1) * ntok], 0, ntok)
                      for j in range(3) for c in range(3)]
        else:
            pieces = (
                [(c, atvs[0][:, c, 0:512], 0, 512) for c in range(3)]
                + [(3 + c, atvs[0][:, c, 512:768], 0, 256) for c in range(3)]
                + [(3 + c, atvs[1][:, c, 0:256], 256, 512) for c in range(3)]
                + [(6 + c, atvs[1][:, c, 256:768], 0, 512) for c in range(3)]
            )
            # full-range ops must open and close the accumulation group
            pieces = pieces[0:3] + pieces[3:9] + pieces[9:12]
        for cc in range(CC):
            ps = pspj.tile([128, 512], F32, tag="pjps")
            for i, (ch, rhs, a, bnd) in enumerate(pieces):
                nc.tensor.matmul(
                    out=ps[:, a:bnd],
                    lhsT=wT[:, ch * 1024 + cc * 128: ch * 1024 + cc * 128 + 128],
                    rhs=rhs,
                    start=(i == 0),
                    stop=(i == len(pieces) - 1),
                )
            nc.scalar.copy(out=dstT[:, cc * ntok:(cc + 1) * ntok],
                           in_=ps[:, 0:ntok])

    # ---- prologue: hq for all 16 b (256 ques tokens) ----
    atq = gather(itq[:], 256)
    project(qwT, atq, hqT, 256)

    hqv = hqT[:].rearrange("p (c t) -> p c t", c=CC)  # [128, 8, 256]
    h2av = h2aT[:].rearrange("p (c h) -> p c h", c=CC)  # [128, 8, 8]
    pv = POOL[:].rearrange("p (c b h) -> p c b h", c=CC, b=BPC)

    # ---- per pair of batch elements ----
    for bp in range(BPC // 2):
        atk = gather(itk[:, bp * 96:(bp + 1) * 96], 512)

        hsT = hstp.tile([128, CC * 512], BF16, tag="hsT")
        project(kwT, atk, hsT, 512)

        for half in range(2):
            b = bp * 2 + half
            hb = half * 256

            # hs token-major bf16: [s-chunk partitions, col st*1024 + c]
            hs_sb = hsbp.tile([128, 2 * 1024], BF16, tag="hs_sb")
            for st in range(2):
                ps = pstr.tile([128, 1024], BF16, tag="trps")
                for cc in range(CC):
                    nc.tensor.transpose(
                        out=ps[:, cc * 128:(cc + 1) * 128],
                        in_=hsT[:, cc * 512 + hb + st * 128: cc * 512 + hb + st * 128 + 128],
                        identity=idbf[:],
                    )
                nc.scalar.copy(out=hs_sb[:, st * 1024:(st + 1) * 1024], in_=ps[:])

            # X^T[c, h*16+q] = hqT[c, q] * h2aT[c, h]
            XT = xtp.tile([128, 1024], BF16, tag="XT")
            nc.vector.tensor_tensor(
                out=XT[:].rearrange("p (c h q) -> p c h q", c=CC, h=H),
                in0=hqv[:, :, b * 16: b * 16 + 16].unsqueeze(2).to_broadcast(
                    [128, CC, H, 16]),
                in1=h2av[:, :, :].unsqueeze(3).to_broadcast([128, CC, H, 16]),
                op=ALU.mult,
            )

            # logits[hq=128, s=256]
            plg = pspj.tile([128, 512], F32, tag="pjps")
            for cc in range(CC):
                nc.tensor.matmul(
                    out=plg[:, 0:256],
                    lhsT=XT[:, cc * 128: cc * 128 + 128],
                    rhs=hsT[:, cc * 512 + hb: cc * 512 + hb + 256],
                    start=(cc == 0),
                    stop=(cc == CC - 1),
                )

            # softmax over flat (q,s) per (b,h); logits tiny -> skip max-sub
            att = attp.tile([128, 256], BF16, tag="att")
            qsum = smlp.tile([128, 1], F32, tag="qsum")
            nc.scalar.activation(att[:], plg[:, 0:256], AF.Exp, accum_out=qsum[:])

            dps = psyt.tile([128, 512], F32, tag="ytps", name="dps")
            nc.tensor.matmul(out=dps[0:8, 0:1], lhsT=sel1[:], rhs=qsum[:],
                             start=True, stop=True)
            r8 = smlp.tile([8, 1], F32, tag="r8")
            nc.vector.reciprocal(r8[:], dps[0:8, 0:1])
            nc.tensor.matmul(out=dps[:, 1:2], lhsT=sel2[:], rhs=r8[:],
                             start=True, stop=True)
            rsb = smlp.tile([128, 1], F32, tag="rsb")
            nc.vector.tensor_copy(rsb[:], dps[:, 1:2])

            attn = attp.tile([128, 256], BF16, tag="attn")
            nc.vector.tensor_scalar_mul(attn[:], att[:], rsb[:])

            # attT [s, hq] bf16
            attT = attp.tile([128, 256], BF16, tag="attT")
            psTb = pstr.tile([128, 256], BF16, tag="trps", name="psTb")
            for st in range(2):
                nc.tensor.transpose(
                    out=psTb[:, st * 128:(st + 1) * 128],
                    in_=attn[:, st * 128:(st + 1) * 128],
                    identity=idbf[:],
                )
            nc.vector.tensor_copy(attT[:], psTb[:])

            # YT[c, hq] per c-chunk; pooled[h,c] = sum_q hqT * sum_s attT*hs
            for ccg in range(2):
                py = psyt.tile([128, 512], F32, tag="ytps")
                for i in range(4):
                    cc = ccg * 4 + i
                    for st in range(2):
                        nc.tensor.matmul(
                            out=py[:, i * 128:(i + 1) * 128],
                            lhsT=hs_sb[:, st * 1024 + cc * 128: st * 1024 + cc * 128 + 128],
                            rhs=attT[:, st * 128:(st + 1) * 128],
                            start=(st == 0),
                            stop=(st == 1),
                        )
                tmp = tmpp.tile([128, 512], F32, tag="tmp")
                nc.vector.tensor_tensor(
                    out=tmp[:].rearrange("p (c h q) -> p c h q", c=4, h=H),
                    in0=py[:].rearrange("p (c h q) -> p c h q", c=4, h=H),
                    in1=hqv[:, ccg * 4:(ccg + 1) * 4, b * 16: b * 16 + 16].unsqueeze(2).to_broadcast([128, 4, H, 16]),
                    op=ALU.mult,
                )
                nc.vector.reduce_sum(
                    out=pv[:, ccg * 4:(ccg + 1) * 4, b, :],
                    in_=tmp[:].rearrange("p (c h q) -> p c h q", c=4, h=H),
                    axis=AX.X,
                )

    # ---- fc: out[o, b] = sum_{h,c} fc_w[o, h*1024+c] * pooled ----
    nc.vector.tensor_copy(POOLb[:], POOL[:])
    poolv = POOLb[:].rearrange("p (c b h) -> p c b h", c=CC, b=BPC)
    pfc = [pspj.tile([128, 512], F32, tag="pjps", name="pfc0"),
           pstr.tile([128, 512], F32, tag="trps", name="pfc1"),
           psyt.tile([128, 512], F32, tag="ytps", name="pfc2")]
    nhc = H * CC
    for h in range(H):
        for cc in range(CC):
            i = h * CC + cc
            fw = fcw_sb[i // FCC]
            fo = (i % FCC) * OUT
            for oc in range(3):
                ocn = OCN[oc]
                nc.tensor.matmul(
                    out=pfc[oc][0:ocn, 0:16],
                    lhsT=fw[:, fo + oc * 128: fo + oc * 128 + ocn],
                    rhs=poolv[:, cc, :, h],
                    start=(i == 0),
                    stop=(i == nhc - 1),
                )
    for oc in range(3):
        ocn = OCN[oc]
        nc.scalar.activation(
            out=fcout[0:ocn, oc * 16: oc * 16 + 16],
            in_=pfc[oc][0:ocn, 0:16],
            func=AF.Identity,
            bias=fcb[0:ocn, oc: oc + 1],
        )

    # ---- sim = fcout.T @ gloveT ; log_softmax over NA ----
    a0 = 0
    for ci, n in enumerate(SIMCH):
        pss = psyt.tile([16, 512], F32, tag="ytps", name="pss")
        for oc in range(3):
            ocn = OCN[oc]
            nc.tensor.matmul(
                out=pss[0:16, 0:n],
                lhsT=fcout[0:ocn, oc * 16: oc * 16 + 16],
                rhs=glo_sb[oc][0:ocn, a0: a0 + n],
                start=(oc == 0),
                stop=(oc == 2),
            )
        junk = tmpp.tile([128, 512], F32, tag="tmp")
        nc.scalar.activation(junk[0:16, 0:n], pss[0:16, 0:n], AF.Exp,
                             accum_out=parti[:, ci: ci + 1])
        nc.vector.tensor_copy(sim_sb[:, a0: a0 + n], pss[0:16, 0:n])
        a0 += n

    nc.vector.reduce_sum(out=tot[:], in_=parti[:, 0:10], axis=AX.X)
    nc.scalar.activation(lse[:], tot[:], AF.Ln)
    # chunked subtract + store so the output DMA overlaps the DVE work
    a0 = 0
    for n in SIMCH:
        nc.vector.tensor_scalar_sub(sim_sb[:, a0:a0 + n],
                                    sim_sb[:, a0:a0 + n], lse[:])
        nc.sync.dma_start(out_d[:, a0:a0 + n], sim_sb[:, a0:a0 + n])
        a0 += n


def _build():
    nc = bacc.Bacc("TRN2", target_bir_lowering=False, debug=False,
                   num_devices=NCORES)
    ins = {}

    def di(name, shape, dtype):
        ins[name] = nc.dram_tensor(name, list(shape), dtype,
                                   kind="ExternalInput").ap()

    di("emb", (UMAX, ES), BF16)
    di("wb", (128, 53982), BF16)
    outs = {"out": nc.dram_tensor("out", [BPC, NA], F32,
                                  kind="ExternalOutput").ap()}

    with tile.TileContext(nc) as tc, ExitStack() as ctx:
        _emit(ctx, tc, ins, outs)
    nc.compile()
    return nc


def _pack_wT(W, bias):
    """[C, 900] f32 -> [128, NCH*1024] bf16 with bias folded at (0,2) row 44."""
    bf = ml_dtypes.bfloat16
    P = np.zeros((128, NCH * 1024), np.float32)
    for j in range(NODES):
        for c in range(3):
            ch = j * 3 + c
            kr = min(128, E - c * 128)
            P[0:kr, ch * 1024:(ch + 1) * 1024] = \
                W[:, j * E + c * 128: j * E + c * 128 + kr].T
    P[44, 2 * 1024:3 * 1024] = bias      # chunk (0,2) row 44 <- ones column
    return P.astype(bf)


def make_in_maps(he_ques, he_kg, emb, q2h_w, q2h_b, k2h_w, k2h_b,
                 h2att_w, h2att_b, fc_w, fc_b, glove_cands):
    f32 = np.float32
    bf = ml_dtypes.bfloat16
    emb = np.asarray(emb, f32)
    he_kg = np.asarray(he_kg).astype(np.int64)
    he_ques = np.asarray(he_ques).astype(np.int64)

    kwT = _pack_wT(np.asarray(k2h_w, f32), np.asarray(k2h_b, f32))
    qwT = _pack_wT(np.asarray(q2h_w, f32), np.asarray(q2h_b, f32))

    h2aT = np.zeros((128, CC * H), f32)
    for cc in range(CC):
        h2aT[:, cc * H:(cc + 1) * H] = np.asarray(h2att_w, f32)[:, cc * 128:(cc + 1) * 128].T

    fcb = np.zeros((128, 3), f32)
    fcb_src = np.asarray(fc_b, f32)
    for oc in range(3):
        fcb[0:OCN[oc], oc] = fcb_src[oc * 128: oc * 128 + OCN[oc]]

    sel1 = np.zeros((128, H), f32)
    for p in range(128):
        sel1[p, p // 16] = 1.0
    sel2 = np.ascontiguousarray(sel1.T)
    idbf = np.eye(128, dtype=bf)

    fcw = np.asarray(fc_w, f32).reshape(OUT, H, CC, 128)
    fcwT = np.ascontiguousarray(
        fcw.transpose(3, 1, 2, 0).reshape(128, H * CC * OUT)).astype(bf)

    glo = np.asarray(glove_cands, f32)
    gloT = np.zeros((3, 128, NA), f32)
    for oc in range(3):
        gloT[oc, 0:OCN[oc], :] = glo[:, oc * 128: oc * 128 + OCN[oc]].T
    gloT = gloT.astype(bf)

    # merged bf16 buffer: qwT | kwT | idbf | fcwT | glo0..2 | f32 consts
    # (bitcast) | per-core i16 idx appended later
    wf = np.zeros((128, 203), f32)
    wf[:, 0:64] = h2aT
    wf[:, 64:67] = fcb
    wf[:, 67:75] = sel1
    wf[0:H, 75:203] = sel2
    wb_common = np.ascontiguousarray(np.concatenate(
        [qwT, kwT, idbf, fcwT, gloT[0], gloT[1], gloT[2],
         np.ascontiguousarray(wf).view(bf)], axis=1))
    shared = {}

    def wrap_idx(flat):
        """[n] -> [128, n//16] int16 wrapped in 16 partitions, replicated."""
        n = flat.shape[0]
        t = np.zeros((128, n // 16), np.int16)
        t[0:16] = flat.reshape(n // 16, 16).T
        for g in range(1, 8):
            t[g * 16:(g + 1) * 16] = t[0:16]
        return t

    maps = []
    for core in range(NCORES):
        kg = he_kg[core * BPC:(core + 1) * BPC]       # [16, 256, 3]
        qu = he_ques[core * BPC:(core + 1) * BPC]     # [16, 16, 3]
        uniq, inv = np.unique(np.concatenate([kg.ravel(), qu.ravel()]),
                              return_inverse=True)
        assert len(uniq) <= UMAX
        kg_c = inv[:kg.size].reshape(kg.shape)
        qu_c = inv[kg.size:].reshape(qu.shape)

        emb_c = np.zeros((UMAX, ES), bf)
        emb_c[0:len(uniq), 0:E] = emb[uniq].astype(bf)
        emb_c[0:len(uniq), E] = bf(1.0)

        # kg idx per pair: i = j*512 + (half*256 + s)
        kg_flat = np.zeros((8, NODES * 512), np.int64)
        for bp in range(8):
            blk = kg_c[2 * bp:2 * bp + 2]             # [2, 256, 3]
            kg_flat[bp] = blk.transpose(2, 0, 1).reshape(NODES, 512).reshape(-1)
        kg_idx = np.concatenate([wrap_idx(kg_flat[bp]) for bp in range(8)],
                                axis=1).astype(np.int16)

        # q idx: i = j*256 + (b*16 + q)
        q_flat = qu_c.transpose(2, 0, 1).reshape(-1)
        q_idx = wrap_idx(q_flat)

        wi = np.ascontiguousarray(
            np.concatenate([q_idx, kg_idx], axis=1).astype(np.int16))

        m = dict(shared)
        m["emb"] = emb_c
        m["wb"] = np.ascontiguousarray(
            np.concatenate([wb_common, wi.view(bf)], axis=1))
        maps.append(m)
    return maps


def kernel(**inputs):
    global _CACHED
    if _CACHED is None:
        _CACHED = _build()
    nc = _CACHED
    in_maps = make_in_maps(**inputs)
    res = run_bass_kernel_spmd(nc, in_maps, list(range(NCORES)))
    return np.concatenate([r["out"] for r in res.results], axis=0)
